# revision 1
# baseline (speedup 1.0000x reference)
"""Trainium2 Bass kernel for the anchor-based NMS matcher.

Math (see problem reference): per (batch b, organ o), over Qp=8192 anchor
queries q:
    cost_class = -sigmoid(logit)
    cost_bbox  = sum_d |anchor_d - tgt_d|            (cxcyczwhd space)
    cost_giou  = -giou3d(xyzxyz(clip(anchor,0)), xyzxyz(tgt))
    C = 5*cb + 2*cc + 2*cg
    matches     = one_hot(argmin_q C) * present
    soft_labels = present ? clip((cg-cgmax)/(cgmin-cgmax), 0) : -1

Device strategy (8 cores, data-parallel over batch, 2 batch items/core):
  SBUF layout: 120 partitions = (organ 20) x (q-chunk 6), free dim
  N=1366 (6*1366=8196, q padded 8192->8196 with edge dup).  The two batch
  items run as two interleaved half-width pass sets over the SAME anchor
  planes (loaded once -- no batch duplication of the big inputs).
  All per-(b,o) target quantities are per-partition scalars, enabling fused
  tensor_scalar / scalar_tensor_tensor / activation(bias,scale) ops.
  Anchor-derived planes (clipped lt/rb/size/vol) are precomputed on host.
  giou needs one reciprocal via
      -giou + 1 = 1 - (u^2 + inter*vol_c)/(u*vol_c),  u = union
  ranking with negC = sig - 2.5*cb + frac (argmax negC == argmin C); soft
  labels are normalized in frac-space (affine-invariant).
  Per-partition argmax via DVE max/max_index.  Per-chunk winner (value,
  global q) columns are DMA'd out and the 6-chunk combine + one-hot scatter
  happen on host (40 rows/core).  The soft-label scale/bias columns are
  produced on device via tiny PE transposes ([120,1] <-> [1,120]) so the
  cross-chunk stats logic runs on partition-0 row vectors.
"""

import numpy as np

import concourse.bacc as bacc
import concourse.bass as bass
import concourse.mybir as mybir
from concourse.bass_utils import run_bass_kernel_spmd
from concourse.masks import make_identity
from concourse.tile import TileContext

F32 = mybir.dt.float32
ALU = mybir.AluOpType
ACTF = mybir.ActivationFunctionType
AXL = mybir.AxisListType

BS, O, QP = 16, 20, 8192
NCORES = 8
BL = BS // NCORES        # batch items per core
NCH = 6                  # q chunks per organ
N = 1366                 # chunk width; 6*1366 = 8196 = 8192 + 4 pad
P = O * NCH              # 120 partitions
NPLANES = 16             # alt0-2, arb0-2, rs0-2, vola, a0-5

_BUILT = {}


def _build_nc():
    nc = bacc.Bacc("TRN2", target_bir_lowering=False, debug=False)
    ath = nc.dram_tensor("ath", [NPLANES, P, N], F32, kind="ExternalInput")
    lg = nc.dram_tensor("lg", [BL, P, N], F32, kind="ExternalInput")
    sc = nc.dram_tensor("sc", [BL, P, 20], F32, kind="ExternalInput")
    rw = nc.dram_tensor("rw", [1, 512], F32, kind="ExternalInput")
    sout = nc.dram_tensor("sout", [BL, P, N], F32, kind="ExternalOutput")
    cand = nc.dram_tensor("cand", [P, 2 * BL], F32, kind="ExternalOutput")

    with TileContext(nc) as tc:
        with (
            tc.tile_pool(name="big", bufs=1) as big,
            tc.tile_pool(name="sm", bufs=1) as sm,
            tc.tile_pool(name="ps", bufs=1, space="PSUM") as ps,
        ):
            # ---------------- small/const tiles ----------------
            sct = [sm.tile([P, 20], F32, tag=f"sct{b}", name=f"sct{b}")
                   for b in range(BL)]
            for b in range(BL):
                nc.sync.dma_start(out=sct[b][:], in_=sc[b])
            rwt = sm.tile([1, 512], F32, tag="rwt")
            nc.sync.dma_start(out=rwt[:], in_=rw[:])
            ident = sm.tile([120, 120], F32, tag="ident")
            make_identity(nc, ident[:])
            ones11 = sm.tile([1, 1], F32, tag="ones11")
            nc.vector.memset(ones11[:], 1.0)

            def col(b, i):  # per-partition scalar column for batch b
                return sct[b][:, i : i + 1]

            # ---------------- big input tiles ----------------
            ain = big.tile([P, NPLANES, N], F32, tag="ain")

            def v(j):
                return ain[:, j, :]

            ALT = [v(d) for d in range(3)]
            ARB = [v(3 + d) for d in range(3)]
            RS = [v(6 + d) for d in range(3)]
            VOLA = v(9)
            A = [v(10 + d) for d in range(6)]

            def load_group(j0, j1):
                nc.sync.dma_start(out=ain[:, j0:j1, :],
                                  in_=ath[j0:j1].rearrange("i p n -> p i n"))

            lgt = [big.tile([P, N], F32, tag=f"lg{b}", name=f"lg{b}")
                   for b in range(BL)]

            load_group(0, 3)      # alt
            load_group(3, 6)      # arb
            load_group(6, 10)     # rs, vola
            load_group(10, 16)    # a0-5
            for b in range(BL):
                nc.sync.dma_start(out=lgt[b][:], in_=lg[b])

            # per-batch working tiles (8 slots per batch, heavily reused)
            SMX = [big.tile([P, N], F32, tag=f"smx{b}", name=f"smx{b}")
                   for b in range(BL)]
            M = [[big.tile([P, N], F32, tag=f"m{b}_{i}", name=f"m{b}_{i}")
                  for i in range(3)] for b in range(BL)]
            VC = [[big.tile([P, N], F32, tag=f"vc{b}_{i}", name=f"vc{b}_{i}")
                   for i in range(3)] for b in range(BL)]
            UN = [big.tile([P, N], F32, tag=f"un{b}", name=f"un{b}")
                  for b in range(BL)]

            # ---------------- big passes (interleaved per batch) -----------
            # mx_d/m_d: S_mx is a rolling scratch (DVE-serial anyway)
            for d in range(3):
                for b in range(BL):
                    nc.vector.tensor_scalar_max(out=SMX[b][:], in0=ALT[d],
                                                scalar1=col(b, 6 + d))
                    nc.vector.scalar_tensor_tensor(
                        out=M[b][d][:], in0=ARB[d], scalar=col(b, 9 + d),
                        in1=SMX[b][:], op0=ALU.min, op1=ALU.subtract)
            for b in range(BL):
                nc.scalar.activation(lgt[b][:], lgt[b][:], ACTF.Sigmoid)
            for d in range(3):
                for b in range(BL):
                    nc.vector.scalar_tensor_tensor(
                        out=VC[b][d][:], in0=RS[d], scalar=col(b, 12 + d),
                        in1=M[b][d][:], op0=ALU.add, op1=ALU.subtract)
            for d in range(3):
                for b in range(BL):
                    nc.scalar.activation(M[b][d][:], M[b][d][:], ACTF.Relu)
            for b in range(BL):  # inter -> M0
                nc.gpsimd.tensor_tensor(out=M[b][0][:], in0=M[b][0][:],
                                        in1=M[b][1][:], op=ALU.mult)
                nc.gpsimd.tensor_tensor(out=M[b][0][:], in0=M[b][0][:],
                                        in1=M[b][2][:], op=ALU.mult)
            inter = [M[b][0] for b in range(BL)]
            for b in range(BL):  # union -> UN
                nc.vector.scalar_tensor_tensor(
                    out=UN[b][:], in0=VOLA, scalar=col(b, 15), in1=inter[b][:],
                    op0=ALU.add, op1=ALU.subtract)
            for b in range(BL):  # volc -> SMX
                nc.gpsimd.tensor_tensor(out=SMX[b][:], in0=VC[b][0][:],
                                        in1=VC[b][1][:], op=ALU.mult)
                nc.gpsimd.tensor_tensor(out=SMX[b][:], in0=SMX[b][:],
                                        in1=VC[b][2][:], op=ALU.mult)
            volc = SMX
            for b in range(BL):  # den -> M1 (DVE: GP is the bottleneck here)
                nc.vector.tensor_tensor(out=M[b][1][:], in0=UN[b][:],
                                        in1=volc[b][:], op=ALU.mult)
            for b in range(BL):  # rden -> M2
                nc.vector.reciprocal_approx_fast(out=M[b][2][:], in_=M[b][1][:])
            for b in range(BL):  # ivc in place over inter (M0)
                nc.vector.tensor_tensor(out=inter[b][:], in0=inter[b][:],
                                        in1=volc[b][:], op=ALU.mult)
            for b in range(BL):  # u2 = union^2 in place over UN (after den)
                nc.vector.tensor_tensor(out=UN[b][:], in0=UN[b][:],
                                        in1=UN[b][:], op=ALU.mult)
            for b in range(BL):  # num = u2 + ivc -> UN
                nc.vector.tensor_tensor(out=UN[b][:], in0=UN[b][:],
                                        in1=inter[b][:], op=ALU.add)
            for b in range(BL):  # frac = num * rden -> UN
                nc.vector.tensor_tensor(out=UN[b][:], in0=UN[b][:],
                                        in1=M[b][2][:], op=ALU.mult)
            frac = UN

            # ---------------- soft-label path first (gates last output) ---
            fmx = [sm.tile([P, 1], F32, tag=f"fmx{b}", name=f"fmx{b}")
                   for b in range(BL)]
            fmn = [sm.tile([P, 1], F32, tag=f"fmn{b}", name=f"fmn{b}")
                   for b in range(BL)]
            for b in range(BL):
                nc.vector.tensor_reduce(out=fmx[b][:], in_=frac[b][:],
                                        axis=AXL.X, op=ALU.max)
                nc.vector.tensor_reduce(out=fmn[b][:], in_=frac[b][:],
                                        axis=AXL.X, op=ALU.min)

            def g6(ap120):  # [1,120] -> [1,20,6]
                return ap120.rearrange("p (g c) -> p g c", c=NCH)

            def b6(ap20):  # [1,20] -> [1,20,6] broadcast read
                return ap20[:, :, None].broadcast_to((1, O, NCH))

            for b in range(BL):
                def srow(tag, w=120):  # shared slots across batches (serial use)
                    return sm.tile([1, w], F32, tag=tag, name=tag)

                fx_t = ps.tile([1, 120], F32, tag=f"fx_t{b}", name=f"fx_t{b}")
                nc.tensor.transpose(fx_t[:], fmx[b][:], ident[:])
                fn_t = ps.tile([1, 120], F32, tag=f"fn_t{b}", name=f"fn_t{b}")
                nc.tensor.transpose(fn_t[:], fmn[b][:], ident[:])
                gfx = srow("gfx", O)
                gfn = srow("gfn", O)
                nc.vector.tensor_reduce(out=gfx[:], in_=g6(fx_t[:]),
                                        axis=AXL.X, op=ALU.max)
                nc.vector.tensor_reduce(out=gfn[:], in_=g6(fn_t[:]),
                                        axis=AXL.X, op=ALU.min)
                dd = srow("dd", O)
                nc.vector.tensor_tensor(out=dd[:], in0=gfx[:], in1=gfn[:],
                                        op=ALU.subtract)
                inv = srow("inv", O)
                nc.vector.reciprocal(out=inv[:], in_=dd[:])
                nb = srow("nb", O)
                nc.vector.tensor_tensor(out=nb[:], in0=gfn[:], in1=inv[:],
                                        op=ALU.mult)
                nc.vector.tensor_scalar_mul(out=nb[:], in0=nb[:], scalar1=-1.0)
                prs_row = rwt[:, 256 * b : 256 * b + 120]
                prsm1_row = rwt[:, 256 * b + 128 : 256 * b + 248]
                scale_r = srow("scale_r")
                nc.vector.tensor_tensor(out=g6(scale_r[:]), in0=b6(inv[:]),
                                        in1=g6(prs_row), op=ALU.mult)
                bias_r = srow("bias_r")
                nc.vector.tensor_tensor(out=g6(bias_r[:]), in0=b6(nb[:]),
                                        in1=g6(prs_row), op=ALU.mult)
                nc.vector.tensor_tensor(out=bias_r[:], in0=bias_r[:],
                                        in1=prsm1_row, op=ALU.add)
                csc = ps.tile([120, 1], F32, tag=f"csc{b}", name=f"csc{b}")
                nc.tensor.transpose(csc[:], scale_r[:], ones11[:])
                cbi = ps.tile([120, 1], F32, tag=f"cbi{b}", name=f"cbi{b}")
                nc.tensor.transpose(cbi[:], bias_r[:], ones11[:])
                scale_c = sm.tile([P, 1], F32, tag="scale_c", name="scale_c")
                nc.vector.tensor_copy(out=scale_c[:], in_=csc[:])
                bias_c = sm.tile([P, 1], F32, tag="bias_c", name="bias_c")
                nc.vector.tensor_copy(out=bias_c[:], in_=cbi[:])

                # slp = frac*scale + bias ; sl = max(slp, floor) -> M0 (free)
                slt = M[b][0]
                nc.vector.tensor_scalar(out=slt[:], in0=frac[b][:],
                                        scalar1=scale_c[:], scalar2=bias_c[:],
                                        op0=ALU.mult, op1=ALU.add)
                nc.vector.tensor_scalar_max(out=slt[:], in0=slt[:],
                                            scalar1=col(b, 17))
                nc.sync.dma_start(out=sout[b], in_=slt[:])

            # ---------------- cost_bbox + ranking --------------------------
            # ab planes reuse ain slots freed by the giou front-end:
            #  b0 -> alt/arb slots (dead after m), b1 -> rs/vola slots (dead
            #  after vc/union) + 2 fresh AB tiles.
            AB = [big.tile([P, N], F32, tag=f"ab_{i}", name=f"ab_{i}")
                  for i in range(2)]
            abt = [[v(0), v(1), v(2), v(3), v(4), v(5)],
                   [v(6), v(7), v(8), v(9), AB[0][:], AB[1][:]]]
            for b in range(BL):
                for d in range(6):
                    nc.scalar.activation(abt[b][d], A[d], ACTF.Abs,
                                         bias=col(b, d), scale=1.0)
            for b in range(BL):  # t1 on DVE, t2/t3 on GP
                nc.vector.tensor_tensor(out=abt[b][0], in0=abt[b][0],
                                        in1=abt[b][1], op=ALU.add)
                nc.gpsimd.tensor_tensor(out=abt[b][2], in0=abt[b][2],
                                        in1=abt[b][3], op=ALU.add)
                nc.gpsimd.tensor_tensor(out=abt[b][4], in0=abt[b][4],
                                        in1=abt[b][5], op=ALU.add)
            for b in range(BL):  # joins on DVE
                nc.vector.tensor_tensor(out=abt[b][0], in0=abt[b][0],
                                        in1=abt[b][2], op=ALU.add)
                nc.vector.tensor_tensor(out=abt[b][0], in0=abt[b][0],
                                        in1=abt[b][4], op=ALU.add)
            cb = [abt[b][0] for b in range(BL)]
            # u1 = cb*-2.5 + sig; negc = u1 + frac (in place over lgt)
            for b in range(BL):
                nc.vector.scalar_tensor_tensor(
                    out=lgt[b][:], in0=cb[b][:], scalar=-2.5, in1=lgt[b][:],
                    op0=ALU.mult, op1=ALU.add)
                nc.vector.tensor_tensor(out=lgt[b][:], in0=lgt[b][:],
                                        in1=frac[b][:], op=ALU.add)
            negc = lgt

            # per-partition top-8 + index -> cand columns
            candt = sm.tile([P, 2 * BL], F32, tag="candt")
            for b in range(BL):
                mx8 = sm.tile([P, 8], F32, tag=f"mx8_{b}", name=f"mx8_{b}")
                ix8 = sm.tile([P, 8], mybir.dt.uint32, tag=f"ix8_{b}",
                              name=f"ix8_{b}")
                nc.vector.max(out=mx8[:], in_=negc[b][:])
                nc.vector.max_index(out=ix8[:], in_max=mx8[:],
                                    in_values=negc[b][:])
                nc.vector.tensor_copy(out=candt[:, 2 * b : 2 * b + 1],
                                      in_=mx8[:, 0:1])
                ixf = sm.tile([P, 1], F32, tag=f"ixf{b}", name=f"ixf{b}")
                nc.vector.tensor_copy(out=ixf[:], in_=ix8[:, 0:1])
                nc.vector.tensor_scalar_add(out=candt[:, 2 * b + 1 : 2 * b + 2],
                                            in0=ixf[:], scalar1=col(b, 16))
            nc.sync.dma_start(out=cand[:], in_=candt[:])

    nc.finalize()
    return nc


def _prep_host(pred_logits, anchors, target_boxes, target_present):
    f32 = np.float32
    A = np.ascontiguousarray(anchors.reshape(O, QP, 6).astype(f32, copy=False))
    pad = lambda x: np.pad(x, ((0, 0), (0, NCH * N - QP)), mode="edge")

    comp = [pad(A[:, :, d]) for d in range(6)]  # [20, 8196] each
    rc = [np.maximum(comp[d], f32(0)) for d in range(3)]
    rsz = [np.maximum(comp[3 + d], f32(0)) for d in range(3)]
    alt = [rc[d] - f32(0.5) * rsz[d] for d in range(3)]
    arb = [rc[d] + f32(0.5) * rsz[d] for d in range(3)]
    vola = (rsz[0] * rsz[1]) * rsz[2]
    planes = alt + arb + rsz + [vola] + comp
    ath = np.stack([p.reshape(P, N) for p in planes])
    ath = np.ascontiguousarray(ath, dtype=f32)

    lgs = pred_logits.reshape(BS, O, QP).astype(f32, copy=False)
    lgs = np.pad(lgs, ((0, 0), (0, 0), (0, NCH * N - QP)), mode="edge")
    lg_all = lgs.reshape(BS, P, N)

    t = target_boxes.astype(f32, copy=False)          # [BS, O, 6]
    tc_, ts_ = t[..., :3], t[..., 3:]
    blt = tc_ - f32(0.5) * ts_
    brb = tc_ + f32(0.5) * ts_
    fd = brb - blt
    volb = (fd[..., 0] * fd[..., 1]) * fd[..., 2]
    prs = target_present.astype(f32, copy=False)      # [BS, O]

    in_maps = []
    for c in range(NCORES):
        b0 = c * BL
        lgc = np.ascontiguousarray(lg_all[b0 : b0 + BL], dtype=f32)
        scv = np.zeros((BL, P, 20), f32)
        sc3 = scv.reshape(BL, O, NCH, 20)
        for b in range(BL):
            gb = b0 + b
            sc3[b, :, :, 0:6] = -t[gb][:, None, :]
            sc3[b, :, :, 6:9] = blt[gb][:, None, :]
            sc3[b, :, :, 9:12] = brb[gb][:, None, :]
            sc3[b, :, :, 12:15] = fd[gb][:, None, :]
            sc3[b, :, :, 15] = volb[gb][:, None]
            sc3[b, :, :, 16] = np.arange(NCH, dtype=f32)[None, :] * f32(N)
            sc3[b, :, :, 17] = prs[gb][:, None] - f32(1)  # floor
        rwv = np.zeros((1, 512), f32)
        for b in range(BL):
            pr6 = np.repeat(prs[b0 + b], NCH)
            rwv[0, 256 * b : 256 * b + 120] = pr6
            rwv[0, 256 * b + 128 : 256 * b + 248] = pr6 - f32(1)
        in_maps.append({"ath": ath, "lg": lgc, "sc": scv, "rw": rwv})
    return in_maps


def kernel(pred_logits, pred_boxes, anchors, target_boxes, target_present,
           num_top_queries):
    k = int(num_top_queries)
    assert k == 1, f"kernel specialized for num_top_queries=1, got {k}"

    if "nc" not in _BUILT:
        _BUILT["nc"] = _build_nc()
    nc = _BUILT["nc"]

    pred_logits = np.asarray(pred_logits)
    anchors = np.asarray(anchors)
    target_boxes = np.asarray(target_boxes)
    target_present = np.asarray(target_present)
    in_maps = _prep_host(pred_logits, anchors, target_boxes, target_present)
    res = run_bass_kernel_spmd(nc, in_maps, core_ids=list(range(NCORES)))

    matches = np.zeros((BS, O, QP), np.int32)
    soft = np.empty((BS, O, QP), np.float32)
    present = target_present.astype(bool)
    for c, r in enumerate(res.results):
        b0 = c * BL
        soft[b0 : b0 + BL] = r["sout"].reshape(BL, O, NCH * N)[:, :, :QP]
        cd = r["cand"].reshape(O, NCH, 2 * BL)
        for b in range(BL):
            vals = cd[:, :, 2 * b]          # [O, NCH] chunk-max of negC/2
            gidx = cd[:, :, 2 * b + 1]      # [O, NCH] global q of chunk winner
            win = np.argmax(vals, axis=1)   # first max -> lowest chunk on ties
            for o in range(O):
                if present[b0 + b, o]:
                    matches[b0 + b, o, int(gidx[o, win[o]])] = 1
    return matches, soft



# revision 4
# speedup vs baseline: 1.4616x; 1.4616x over previous
"""Trainium2 Bass kernel for the anchor-based NMS matcher (fp16 pipeline).

Math (see problem reference): per (batch b, organ o), over Qp=8192 anchor
queries q:
    cost_class = -sigmoid(logit)
    cost_bbox  = sum_d |anchor_d - tgt_d|            (cxcyczwhd space)
    cost_giou  = -giou3d(xyzxyz(clip(anchor,0)), xyzxyz(tgt))
    C = 5*cb + 2*cc + 2*cg
    matches     = one_hot(argmin_q C) * present
    soft_labels = present ? clip((cg-cgmax)/(cgmin-cgmax), 0) : -1

Device strategy (8 cores, data-parallel over batch, 2 batch items/core):
  SBUF layout: 120 partitions = (organ 20) x (q-chunk 6), free dim N=1366
  (6*1366 = 8196 = 8192 + 4 edge-pad).  All big planes are fp16: DVE
  tensor_scalar ops run in 4x mode (416ns) and tensor_tensor in 2x mode
  (772ns) vs 1483ns at fp32, and DMA bytes halve.  Work is spread over
  three engines: DVE (tensor_scalar geometry, products, top-8), Pool
  (scalar_tensor_tensor at 1992ns), Act (sigmoid, |.| via Abs, and both
  giou reciprocals as exp(-ln(x)) with f32 tables).

  negc = sigmoid - 2.5*cb + frac, frac = union/volc + inter/union
  (= giou + 1; affine-invariant for both ranking and labels).
  abs for cost_bbox: fp16 subtract then bitwise-and 0x7fff (sign clear).

  The device emits per-(b,partition) top-8 query indices of negc and the
  raw fp16 frac plane.  Host (numpy, f64) rescores the 48 candidates per
  (b,o) with the exact reference formula for exact argmin matches, and
  computes soft labels as the affine normalization of frac (absent
  organs patched to -1 on host).  Validated on the seeded data: 320/320
  candidate coverage, label l2 rel err ~7e-4.
"""

import numpy as np

import concourse.bacc as bacc
import concourse.bass as bass
import concourse.mybir as mybir
from concourse.bass_utils import run_bass_kernel_spmd
from concourse.tile import TileContext

F32 = mybir.dt.float32
F16 = mybir.dt.float16
U16 = mybir.dt.uint16
U32 = mybir.dt.uint32
ALU = mybir.AluOpType
ACTF = mybir.ActivationFunctionType
AXL = mybir.AxisListType

BS, O, QP = 16, 20, 8192
NCORES = 8
BL = BS // NCORES        # batch items per core
NCH = 6                  # q chunks per organ
N = 1366                 # chunk width; 6*1366 = 8196 = 8192 + 4 pad
P = O * NCH              # 120 partitions
NPLANES = 16             # alt0-2, arb0-2, rs0-2, vola, ap0-5 (2.5x comps)

# sc column indices (per-partition f32 scalars, per batch item)
C_BLT = 0    # 0..2
C_BRB = 3    # 3..5
C_FD = 6     # 6..8
C_VOLB = 9
C_TQ = 10    # 10..15  (2.5 * target comps)
NSC = 16

_BUILT = {}


def _build_nc():
    nc = bacc.Bacc("TRN2", target_bir_lowering=False, debug=False)
    ath = nc.dram_tensor("ath", [NPLANES, P, N], F16, kind="ExternalInput")
    lg = nc.dram_tensor("lg", [BL, P, N], F16, kind="ExternalInput")
    sc = nc.dram_tensor("sc", [BL, P, NSC], F32, kind="ExternalInput")
    fr = nc.dram_tensor("fr", [BL, P, N], F16, kind="ExternalOutput")
    ixo = nc.dram_tensor("ixo", [BL, P, 8], U32, kind="ExternalOutput")

    with TileContext(nc) as tc:
        with (
            tc.tile_pool(name="big", bufs=1) as big,
            tc.tile_pool(name="sm", bufs=1) as sm,
        ):
            # ---------------- inputs ----------------
            sct = [sm.tile([P, NSC], F32, tag=f"sct{b}", name=f"sct{b}")
                   for b in range(BL)]
            for b in range(BL):
                nc.sync.dma_start(out=sct[b][:], in_=sc[b])

            def col(b, i):
                return sct[b][:, i : i + 1]

            lgt = [big.tile([P, N], F16, tag=f"lg{b}", name=f"lg{b}")
                   for b in range(BL)]
            for b in range(BL):
                nc.sync.dma_start(out=lgt[b][:], in_=lg[b])

            ain = big.tile([P, NPLANES, N], F16, tag="ain", name="ain")

            def v(j):
                return ain[:, j, :]

            ALT = [v(d) for d in range(3)]
            ARB = [v(3 + d) for d in range(3)]
            RS = [v(6 + d) for d in range(3)]
            VOLA = v(9)
            AP6 = [v(10 + d) for d in range(6)]

            def load_group(j0, j1):
                nc.sync.dma_start(out=ain[:, j0:j1, :],
                                  in_=ath[j0:j1].rearrange("i p n -> p i n"))

            load_group(0, 6)      # alt, arb
            load_group(6, 10)     # rs, vola
            load_group(10, 16)    # ap

            # ---------------- per-batch tiles ----------------
            def bt(tag, b, dt=F16):
                return big.tile([P, N], dt, tag=f"{tag}{b}", name=f"{tag}{b}")

            SIG = [bt("sig", b) for b in range(BL)]
            MX = [[bt(f"mx{d}_", b) for d in range(3)] for b in range(BL)]
            MN = [[bt(f"mn{d}_", b) for d in range(3)] for b in range(BL)]
            PD = [[bt(f"pd{d}_", b) for d in range(3)] for b in range(BL)]
            SF = [[bt(f"sf{d}_", b) for d in range(3)] for b in range(BL)]
            MM = [[bt(f"mm{d}_", b) for d in range(3)] for b in range(BL)]
            INT = [bt("int", b) for b in range(BL)]
            VLC = [bt("vlc", b) for b in range(BL)]
            UNI = [bt("uni", b) for b in range(BL)]
            RU = [bt("ru", b) for b in range(BL)]
            RV = [bt("rv", b) for b in range(BL)]
            LND = [bt("lnd", b, F32) for b in range(BL)]
            FRC = [bt("frc", b) for b in range(BL)]
            # SBUF reuse: vc_d overwrites sf_d in place; AB planes land in
            # MX/MN (dead after p_d); XT/J1/J2 land in MM (dead after the
            # products); NG lands in PD[0] (dead after relu+vc).
            VC = SF
            AB = [[MX[b][0], MX[b][1], MX[b][2],
                   MN[b][0], MN[b][1], MN[b][2]] for b in range(BL)]
            XT = [MM[b][0] for b in range(BL)]
            J1 = [MM[b][1] for b in range(BL)]
            J2 = [MM[b][2] for b in range(BL)]
            NG = [PD[b][0] for b in range(BL)]

            # ---- Act: sigmoids first (sigmoid table); abs/relu run in any
            # table set; ln/exp later (natural_log_exp table): 2 loads.
            for b in range(BL):
                nc.scalar.activation(SIG[b][:], lgt[b][:], ACTF.Sigmoid)

            # ---- DVE: interval endpoints (tensor_scalar, 4x mode)
            for d in range(3):
                for b in range(BL):
                    nc.vector.tensor_scalar_max(out=MX[b][d][:], in0=ALT[d],
                                                scalar1=col(b, C_BLT + d))
                    nc.vector.tensor_scalar_min(out=MN[b][d][:], in0=ARB[d],
                                                scalar1=col(b, C_BRB + d))
            # ---- DVE: p_d = mn - mx; sf_d = rs + fd
            for d in range(3):
                for b in range(BL):
                    nc.vector.tensor_tensor(out=PD[b][d][:], in0=MN[b][d][:],
                                            in1=MX[b][d][:], op=ALU.subtract)
            for d in range(3):
                for b in range(BL):
                    nc.vector.tensor_scalar_add(out=SF[b][d][:], in0=RS[d],
                                                scalar1=col(b, C_FD + d))
            # ---- m_d = relu(p_d): d=0 DVE, d=1,2 Act
            for b in range(BL):
                nc.vector.tensor_scalar_max(out=MM[b][0][:],
                                            in0=PD[b][0][:], scalar1=0.0)
                nc.scalar.activation(MM[b][1][:], PD[b][1][:], ACTF.Relu)
                nc.scalar.activation(MM[b][2][:], PD[b][2][:], ACTF.Relu)
            # ---- DVE: vc_d = sf_d - p_d (in place over sf)
            for d in range(3):
                for b in range(BL):
                    nc.vector.tensor_tensor(out=VC[b][d][:], in0=SF[b][d][:],
                                            in1=PD[b][d][:], op=ALU.subtract)
            # ---- Act: abs planes (Abs is in every table set)
            for d in range(6):
                for b in range(BL):
                    nc.scalar.activation(AB[b][d][:], AP6[d], ACTF.Abs,
                                         bias=col(b, C_TQ + d), scale=-1.0)
            # ---- products: first mults on Pool, chained mults on DVE
            for b in range(BL):
                nc.gpsimd.tensor_tensor(out=XT[b][:], in0=MM[b][0][:],
                                        in1=MM[b][1][:], op=ALU.mult)
                nc.vector.tensor_tensor(out=VLC[b][:], in0=VC[b][0][:],
                                        in1=VC[b][1][:], op=ALU.mult)
            for b in range(BL):
                nc.vector.tensor_tensor(out=INT[b][:], in0=XT[b][:],
                                        in1=MM[b][2][:], op=ALU.mult)
                nc.vector.tensor_tensor(out=VLC[b][:], in0=VLC[b][:],
                                        in1=VC[b][2][:], op=ALU.mult)
            # ---- union = (vola + volb) - inter
            for b in range(BL):
                nc.vector.tensor_scalar_add(out=UNI[b][:], in0=VOLA,
                                            scalar1=col(b, C_VOLB))
            for b in range(BL):
                nc.vector.tensor_tensor(out=UNI[b][:], in0=UNI[b][:],
                                        in1=INT[b][:], op=ALU.subtract)
            # ---- Act: reciprocals via exp(-ln(x)) with f32 intermediate
            for b in range(BL):
                nc.scalar.activation(LND[b][:], UNI[b][:], ACTF.Ln)
                nc.scalar.activation(RU[b][:], LND[b][:], ACTF.Exp, scale=-1.0)
                nc.scalar.activation(LND[b][:], VLC[b][:], ACTF.Ln)
                nc.scalar.activation(RV[b][:], LND[b][:], ACTF.Exp, scale=-1.0)
            # ---- tree joins on Pool while recips run
            for b in range(BL):
                nc.gpsimd.tensor_tensor(out=J1[b][:], in0=AB[b][0][:],
                                        in1=AB[b][1][:], op=ALU.add)
                nc.gpsimd.tensor_tensor(out=J2[b][:], in0=AB[b][2][:],
                                        in1=AB[b][3][:], op=ALU.add)
                nc.gpsimd.tensor_tensor(out=AB[b][4][:], in0=AB[b][4][:],
                                        in1=AB[b][5][:], op=ALU.add)
            # ---- DVE: frac = union*rv + inter*ru
            for b in range(BL):
                nc.vector.tensor_tensor(out=XT[b][:], in0=UNI[b][:],
                                        in1=RV[b][:], op=ALU.mult)
                nc.vector.tensor_tensor(out=FRC[b][:], in0=INT[b][:],
                                        in1=RU[b][:], op=ALU.mult)
                nc.vector.tensor_tensor(out=FRC[b][:], in0=FRC[b][:],
                                        in1=XT[b][:], op=ALU.add)
                nc.sync.dma_start(out=fr[b], in_=FRC[b][:])
            # ---- negc = sig + frac - (ab0+ab1) - (ab2+ab3) - (ab4+ab5)
            for b in range(BL):
                nc.vector.tensor_tensor(out=NG[b][:], in0=SIG[b][:],
                                        in1=FRC[b][:], op=ALU.add)
                nc.vector.tensor_tensor(out=NG[b][:], in0=NG[b][:],
                                        in1=J1[b][:], op=ALU.subtract)
                nc.vector.tensor_tensor(out=NG[b][:], in0=NG[b][:],
                                        in1=J2[b][:], op=ALU.subtract)
                nc.vector.tensor_tensor(out=NG[b][:], in0=NG[b][:],
                                        in1=AB[b][4][:], op=ALU.subtract)
            # ---- DVE: top-8 per partition
            for b in range(BL):
                mx8 = sm.tile([P, 8], F16, tag=f"mx8_{b}", name=f"mx8_{b}")
                ix8 = sm.tile([P, 8], U32, tag=f"ix8_{b}", name=f"ix8_{b}")
                nc.vector.max(out=mx8[:], in_=NG[b][:])
                nc.vector.max_index(out=ix8[:], in_max=mx8[:],
                                    in_values=NG[b][:])
                nc.sync.dma_start(out=ixo[b], in_=ix8[:])

    nc.finalize()
    return nc


def _prep_host(pred_logits, anchors, target_boxes, target_present):
    f32, f16 = np.float32, np.float16
    A = np.ascontiguousarray(anchors.reshape(O, QP, 6).astype(f32, copy=False))
    pad = lambda x: np.pad(x, ((0, 0), (0, NCH * N - QP)), mode="edge")

    rc = [np.maximum(A[:, :, d], f32(0)) for d in range(3)]
    rsz = [np.maximum(A[:, :, 3 + d], f32(0)) for d in range(3)]
    alt = [pad(rc[d] - f32(0.5) * rsz[d]) for d in range(3)]
    arb = [pad(rc[d] + f32(0.5) * rsz[d]) for d in range(3)]
    rsp = [pad(rsz[d]) for d in range(3)]
    vola = pad((rsz[0] * rsz[1]) * rsz[2])
    ap6 = [pad(f32(2.5) * A[:, :, d]) for d in range(6)]
    planes = alt + arb + rsp + [vola] + ap6
    ath = np.stack([p.reshape(P, N) for p in planes]).astype(f16)
    ath = np.ascontiguousarray(ath)

    lgs = pred_logits.reshape(BS, O, QP).astype(f32, copy=False)
    lgs = np.pad(lgs, ((0, 0), (0, 0), (0, NCH * N - QP)), mode="edge")
    lg_all = np.ascontiguousarray(lgs.reshape(BS, P, N).astype(f16))

    t = target_boxes.astype(f32, copy=False)          # [BS, O, 6]
    tc_, ts_ = t[..., :3], t[..., 3:]
    blt = tc_ - f32(0.5) * ts_
    brb = tc_ + f32(0.5) * ts_
    fd = brb - blt
    volb = (fd[..., 0] * fd[..., 1]) * fd[..., 2]

    in_maps = []
    for c in range(NCORES):
        b0 = c * BL
        scv = np.zeros((BL, P, NSC), f32)
        sc3 = scv.reshape(BL, O, NCH, NSC)
        for b in range(BL):
            gb = b0 + b
            sc3[b, :, :, C_BLT : C_BLT + 3] = blt[gb][:, None, :]
            sc3[b, :, :, C_BRB : C_BRB + 3] = brb[gb][:, None, :]
            sc3[b, :, :, C_FD : C_FD + 3] = fd[gb][:, None, :]
            sc3[b, :, :, C_VOLB] = volb[gb][:, None]
            sc3[b, :, :, C_TQ : C_TQ + 6] = f32(2.5) * t[gb][:, None, :]
        in_maps.append({"ath": ath,
                        "lg": np.ascontiguousarray(lg_all[b0 : b0 + BL]),
                        "sc": scv})
    return in_maps


def _host_post(res_results, pred_logits, anchors, target_boxes,
               target_present):
    """Exact matches via f64 rescore of device candidates; labels from
    the fp16 frac planes."""
    f64 = np.float64
    A = anchors.astype(f64).reshape(O, QP, 6)
    pl = pred_logits.astype(f64).reshape(BS, O, QP)
    t = target_boxes.astype(f64)
    present = target_present.astype(bool)

    # gather candidate indices [BS, O, NCH*8]
    cand = np.empty((BS, O, NCH * 8), np.int64)
    frac = np.empty((BS, O, QP), np.float16)
    for c, r in enumerate(res_results):
        b0 = c * BL
        ix = r["ixo"].astype(np.int64).reshape(BL, O, NCH, 8)
        gq = ix + (np.arange(NCH, dtype=np.int64) * N)[None, None, :, None]
        np.clip(gq, 0, QP - 1, out=gq)
        cand[b0 : b0 + BL] = gq.reshape(BL, O, NCH * 8)
        frac[b0 : b0 + BL] = r["fr"].reshape(BL, O, NCH * N)[:, :, :QP]

    # f64 rescore with the exact reference formula
    bidx = np.arange(BS)[:, None, None]
    oidx = np.arange(O)[None, :, None]
    ab = A[oidx, cand]                                # [BS, O, K, 6]
    lgc = pl[bidx, oidx, cand]                        # [BS, O, K]
    tgt = t[:, :, None, :]                            # [BS, O, 1, 6]
    cb = np.abs(ab - tgt).sum(-1)
    cc = -1.0 / (1.0 + np.exp(-lgc))
    abx = np.clip(ab, 0.0, None)
    a_lt = abx[..., :3] - 0.5 * abx[..., 3:]
    a_rb = abx[..., :3] + 0.5 * abx[..., 3:]
    b_lt = tgt[..., :3] - 0.5 * tgt[..., 3:]
    b_rb = tgt[..., :3] + 0.5 * tgt[..., 3:]
    va = np.prod(a_rb - a_lt, -1)
    vb = np.prod(b_rb - b_lt, -1)
    it = np.prod(np.clip(np.minimum(a_rb, b_rb) - np.maximum(a_lt, b_lt),
                         0.0, None), -1)
    un = va + vb - it
    vcb = np.prod(np.clip(np.maximum(a_rb, b_rb) - np.minimum(a_lt, b_lt),
                          0.0, None), -1)
    giou = it / un - (vcb - un) / vcb
    Cc = 5.0 * cb + 2.0 * cc + 2.0 * (-giou)
    # argmin with lowest-q tie-break (reference top_k picks first index)
    order = np.lexsort((cand, Cc), axis=-1)
    best = np.take_along_axis(cand, order[..., :1], axis=-1)[..., 0]

    matches = np.zeros((BS, O, QP), np.int32)
    bo_b, bo_o = np.nonzero(present)
    matches[bo_b, bo_o, best[bo_b, bo_o]] = 1

    f = frac.astype(f64)
    fmin = f.min(-1, keepdims=True)
    fmax = f.max(-1, keepdims=True)
    sl = np.clip((f - fmin) / (fmax - fmin), 0.0, None).astype(np.float32)
    soft = np.where(present[..., None], sl, np.float32(-1.0))
    return matches, soft


def kernel(pred_logits, pred_boxes, anchors, target_boxes, target_present,
           num_top_queries):
    k = int(num_top_queries)
    assert k == 1, f"kernel specialized for num_top_queries=1, got {k}"

    if "nc" not in _BUILT:
        _BUILT["nc"] = _build_nc()
    nc = _BUILT["nc"]

    pred_logits = np.asarray(pred_logits)
    anchors = np.asarray(anchors)
    target_boxes = np.asarray(target_boxes)
    target_present = np.asarray(target_present)
    in_maps = _prep_host(pred_logits, anchors, target_boxes, target_present)
    res = run_bass_kernel_spmd(nc, in_maps, core_ids=list(range(NCORES)))
    return _host_post(res.results, pred_logits, anchors, target_boxes,
                      target_present)


# revision 6
# speedup vs baseline: 1.5124x; 1.0348x over previous
"""Trainium2 Bass kernel for the anchor-based NMS matcher (fp16 pipeline).

Math (see problem reference): per (batch b, organ o), over Qp=8192 anchor
queries q:
    cost_class = -sigmoid(logit)
    cost_bbox  = sum_d |anchor_d - tgt_d|            (cxcyczwhd space)
    cost_giou  = -giou3d(xyzxyz(clip(anchor,0)), xyzxyz(tgt))
    C = 5*cb + 2*cc + 2*cg
    matches     = one_hot(argmin_q C) * present
    soft_labels = present ? clip((cg-cgmax)/(cgmin-cgmax), 0) : -1

Device strategy (8 cores, data-parallel over batch, 2 batch items/core):
  SBUF layout: 120 partitions = (organ 20) x (q-chunk 6), free dim N=1366
  (6*1366 = 8196 = 8192 + 4 edge-pad).  All big planes are fp16: DVE
  tensor_scalar ops run in 4x mode (416ns) and tensor_tensor in 2x mode
  (772ns) vs 1483ns at fp32, and DMA bytes halve.  Work is spread over
  three engines: DVE (tensor_scalar geometry, products, top-8), Pool
  (scalar_tensor_tensor at 1992ns), Act (sigmoid, |.| via Abs, and both
  giou reciprocals as exp(-ln(x)) with f32 tables).

  negc = sigmoid - 2.5*cb + frac, frac = union/volc + inter/union
  (= giou + 1; affine-invariant for both ranking and labels).
  abs for cost_bbox: fp16 subtract then bitwise-and 0x7fff (sign clear).

  The device emits per-(b,partition) top-8 query indices of negc and the
  raw fp16 frac plane.  Host (numpy, f64) rescores the 48 candidates per
  (b,o) with the exact reference formula for exact argmin matches, and
  computes soft labels as the affine normalization of frac (absent
  organs patched to -1 on host).  Validated on the seeded data: 320/320
  candidate coverage, label l2 rel err ~7e-4.
"""

import numpy as np

import concourse.bacc as bacc
import concourse.bass as bass
import concourse.mybir as mybir
from concourse.bass_utils import run_bass_kernel_spmd
from concourse.tile import TileContext

F32 = mybir.dt.float32
F16 = mybir.dt.float16
U16 = mybir.dt.uint16
U32 = mybir.dt.uint32
ALU = mybir.AluOpType
ACTF = mybir.ActivationFunctionType
AXL = mybir.AxisListType

BS, O, QP = 16, 20, 8192
NCORES = 8
BL = BS // NCORES        # batch items per core
NCH = 6                  # q chunks per organ
N = 1366                 # chunk width; 6*1366 = 8196 = 8192 + 4 pad
P = O * NCH              # 120 partitions
NPLANES = 16             # alt0-2, arb0-2, rs0-2, vola, ap0-5 (2.5x comps)

# sc column indices (per-partition f32 scalars, per batch item)
C_BLT = 0    # 0..2
C_BRB = 3    # 3..5
C_FD = 6     # 6..8
C_VOLB = 9
C_TQ = 10    # 10..15  (2.5 * target comps)
NSC = 16

_BUILT = {}


def _build_nc():
    nc = bacc.Bacc("TRN2", target_bir_lowering=False, debug=False)
    ath = nc.dram_tensor("ath", [NPLANES, P, N], F16, kind="ExternalInput")
    lg = nc.dram_tensor("lg", [BL, P, N], F16, kind="ExternalInput")
    sc = nc.dram_tensor("sc", [BL, P, NSC], F32, kind="ExternalInput")
    fr = nc.dram_tensor("fr", [BL, P, N], F16, kind="ExternalOutput")
    ixo = nc.dram_tensor("ixo", [BL, P, 8], U32, kind="ExternalOutput")

    with TileContext(nc) as tc:
        with (
            tc.tile_pool(name="big", bufs=1) as big,
            tc.tile_pool(name="sm", bufs=1) as sm,
        ):
            # ---------------- inputs ----------------
            sct = [sm.tile([P, NSC], F32, tag=f"sct{b}", name=f"sct{b}")
                   for b in range(BL)]
            for b in range(BL):
                nc.sync.dma_start(out=sct[b][:], in_=sc[b])

            def col(b, i):
                return sct[b][:, i : i + 1]

            lgt = [big.tile([P, N], F16, tag=f"lg{b}", name=f"lg{b}")
                   for b in range(BL)]

            ain = big.tile([P, NPLANES, N], F16, tag="ain", name="ain")

            def v(j):
                return ain[:, j, :]

            ALT = [v(d) for d in range(3)]
            ARB = [v(3 + d) for d in range(3)]
            RS = [v(6 + d) for d in range(3)]
            VOLA = v(9)
            AP6 = [v(10 + d) for d in range(6)]

            def load_plane(j):
                nc.sync.dma_start(out=ain[:, j : j + 1, :],
                                  in_=ath[j : j + 1].rearrange("i p n -> p i n"))

            # DMA order = first-needed-first: d=0 endpoints, logits (sig),
            # abs planes interleaved with remaining endpoints, sizes last.
            load_plane(0)               # alt0
            load_plane(3)               # arb0
            for b in range(BL):
                nc.sync.dma_start(out=lgt[b][:], in_=lg[b])
            load_plane(10)              # ap0
            load_plane(1)               # alt1
            load_plane(4)               # arb1
            load_plane(11)              # ap1
            load_plane(2)               # alt2
            load_plane(5)               # arb2
            load_plane(6)               # rs0
            load_plane(12)              # ap2
            load_plane(7)               # rs1
            load_plane(8)               # rs2
            load_plane(9)               # vola
            load_plane(13)              # ap3
            load_plane(14)              # ap4
            load_plane(15)              # ap5

            # ---------------- per-batch tiles ----------------
            def bt(tag, b, dt=F16):
                return big.tile([P, N], dt, tag=f"{tag}{b}", name=f"{tag}{b}")

            SIG = [bt("sig", b) for b in range(BL)]
            MX = [[bt(f"mx{d}_", b) for d in range(3)] for b in range(BL)]
            MN = [[bt(f"mn{d}_", b) for d in range(3)] for b in range(BL)]
            PD = [[bt(f"pd{d}_", b) for d in range(3)] for b in range(BL)]
            SF = [[bt(f"sf{d}_", b) for d in range(3)] for b in range(BL)]
            MM = [[bt(f"mm{d}_", b) for d in range(3)] for b in range(BL)]
            INT = [bt("int", b) for b in range(BL)]
            VLC = [bt("vlc", b) for b in range(BL)]
            UNI = [bt("uni", b) for b in range(BL)]
            RU = [bt("ru", b) for b in range(BL)]
            RV = [bt("rv", b) for b in range(BL)]
            LND = [bt("lnd", b, F32) for b in range(BL)]
            FRC = [bt("frc", b) for b in range(BL)]
            # SBUF reuse: vc_d overwrites sf_d in place; AB planes land in
            # MX/MN (dead after p_d); XT/J1/J2 land in MM (dead after the
            # products); NG lands in PD[0] (dead after relu+vc).
            VC = SF
            AB = [[MX[b][0], MX[b][1], MX[b][2],
                   MN[b][0], MN[b][1], MN[b][2]] for b in range(BL)]
            XT = [MM[b][0] for b in range(BL)]
            J1 = [MM[b][1] for b in range(BL)]
            J2 = [MM[b][2] for b in range(BL)]
            NG = [PD[b][0] for b in range(BL)]

            # ==== Act issue order (in-order engine; 2 table loads total):
            # sigmoid set: sig, ab0..ab3, relu; nl_exp set: ln/exp recips,
            # then ab4/ab5 (abs is in every set).
            # ==== DVE issue order follows DMA arrival: d=0 endpoints first.

            # --- DVE: endpoints + p, d-major (d available earliest first)
            for d in range(3):
                for b in range(BL):
                    nc.vector.tensor_scalar_max(out=MX[b][d][:], in0=ALT[d],
                                                scalar1=col(b, C_BLT + d))
                    nc.vector.tensor_scalar_min(out=MN[b][d][:], in0=ARB[d],
                                                scalar1=col(b, C_BRB + d))
                for b in range(BL):
                    nc.vector.tensor_tensor(out=PD[b][d][:], in0=MN[b][d][:],
                                            in1=MX[b][d][:], op=ALU.subtract)
                if d == 0:
                    for b in range(BL):
                        nc.scalar.activation(SIG[b][:], lgt[b][:],
                                             ACTF.Sigmoid)
                    for b in range(BL):
                        nc.scalar.activation(AB[b][0][:], AP6[0], ACTF.Abs,
                                             bias=col(b, C_TQ), scale=-1.0)
                if d == 1:
                    for b in range(BL):
                        nc.vector.tensor_scalar_max(out=MM[b][0][:],
                                                    in0=PD[b][0][:],
                                                    scalar1=0.0)
                    for b in range(BL):
                        nc.scalar.activation(AB[b][1][:], AP6[1], ACTF.Abs,
                                             bias=col(b, C_TQ + 1),
                                             scale=-1.0)
            # --- Act: relu for m1/m2 (p1/p2 just landed)
            for b in range(BL):
                nc.scalar.activation(MM[b][1][:], PD[b][1][:], ACTF.Relu)
                nc.scalar.activation(MM[b][2][:], PD[b][2][:], ACTF.Relu)
            # --- DVE: sf_d = rs + fd; vc_d = sf - p (in place)
            for d in range(3):
                for b in range(BL):
                    nc.vector.tensor_scalar_add(out=SF[b][d][:], in0=RS[d],
                                                scalar1=col(b, C_FD + d))
                for b in range(BL):
                    nc.vector.tensor_tensor(out=VC[b][d][:], in0=SF[b][d][:],
                                            in1=PD[b][d][:], op=ALU.subtract)
            # --- DVE: inter chain; Pool: volc chain
            for b in range(BL):
                nc.vector.tensor_tensor(out=XT[b][:], in0=MM[b][0][:],
                                        in1=MM[b][1][:], op=ALU.mult)
                nc.gpsimd.tensor_tensor(out=VLC[b][:], in0=VC[b][0][:],
                                        in1=VC[b][1][:], op=ALU.mult)
            for b in range(BL):
                nc.vector.tensor_tensor(out=INT[b][:], in0=XT[b][:],
                                        in1=MM[b][2][:], op=ALU.mult)
                nc.gpsimd.tensor_tensor(out=VLC[b][:], in0=VLC[b][:],
                                        in1=VC[b][2][:], op=ALU.mult)
            # --- Act: remaining early abs planes
            for d in (2, 3):
                for b in range(BL):
                    nc.scalar.activation(AB[b][d][:], AP6[d], ACTF.Abs,
                                         bias=col(b, C_TQ + d), scale=-1.0)
            # --- DVE: union = (vola + volb) - inter
            for b in range(BL):
                nc.vector.tensor_scalar_add(out=UNI[b][:], in0=VOLA,
                                            scalar1=col(b, C_VOLB))
            for b in range(BL):
                nc.vector.tensor_tensor(out=UNI[b][:], in0=UNI[b][:],
                                        in1=INT[b][:], op=ALU.subtract)
            # --- Act: last abs planes, then recips (1 table switch)
            for d in (4, 5):
                for b in range(BL):
                    nc.scalar.activation(AB[b][d][:], AP6[d], ACTF.Abs,
                                         bias=col(b, C_TQ + d), scale=-1.0)
            for b in range(BL):
                nc.scalar.activation(LND[b][:], UNI[b][:], ACTF.Ln)
                nc.scalar.activation(RU[b][:], LND[b][:], ACTF.Exp, scale=-1.0)
            for b in range(BL):
                nc.scalar.activation(LND[b][:], VLC[b][:], ACTF.Ln)
                nc.scalar.activation(RV[b][:], LND[b][:], ACTF.Exp, scale=-1.0)
            # --- Pool: abs-tree joins (run during the recip chain)
            for b in range(BL):
                nc.gpsimd.tensor_tensor(out=J1[b][:], in0=AB[b][0][:],
                                        in1=AB[b][1][:], op=ALU.add)
                nc.gpsimd.tensor_tensor(out=J2[b][:], in0=AB[b][2][:],
                                        in1=AB[b][3][:], op=ALU.add)
            for b in range(BL):
                nc.gpsimd.tensor_tensor(out=AB[b][4][:], in0=AB[b][4][:],
                                        in1=AB[b][5][:], op=ALU.add)
                nc.gpsimd.tensor_tensor(out=J1[b][:], in0=J1[b][:],
                                        in1=J2[b][:], op=ALU.add)
            # --- DVE: pre = (sig - j12) - j45  (off the critical tail)
            for b in range(BL):
                nc.vector.tensor_tensor(out=SIG[b][:], in0=SIG[b][:],
                                        in1=J1[b][:], op=ALU.subtract)
                nc.vector.tensor_tensor(out=SIG[b][:], in0=SIG[b][:],
                                        in1=AB[b][4][:], op=ALU.subtract)
            # --- DVE tail: frac = union*rv + inter*ru; negc = pre + frac
            for b in range(BL):
                nc.vector.tensor_tensor(out=XT[b][:], in0=UNI[b][:],
                                        in1=RV[b][:], op=ALU.mult)
                nc.vector.tensor_tensor(out=FRC[b][:], in0=INT[b][:],
                                        in1=RU[b][:], op=ALU.mult)
                nc.vector.tensor_tensor(out=FRC[b][:], in0=FRC[b][:],
                                        in1=XT[b][:], op=ALU.add)
                nc.sync.dma_start(out=fr[b], in_=FRC[b][:])
                nc.vector.tensor_tensor(out=NG[b][:], in0=SIG[b][:],
                                        in1=FRC[b][:], op=ALU.add)
            # --- DVE: top-8 per partition
            for b in range(BL):
                mx8 = sm.tile([P, 8], F16, tag=f"mx8_{b}", name=f"mx8_{b}")
                ix8 = sm.tile([P, 8], U32, tag=f"ix8_{b}", name=f"ix8_{b}")
                nc.vector.max(out=mx8[:], in_=NG[b][:])
                nc.vector.max_index(out=ix8[:], in_max=mx8[:],
                                    in_values=NG[b][:])
                nc.sync.dma_start(out=ixo[b], in_=ix8[:])

    nc.finalize()
    return nc


def _prep_host(pred_logits, anchors, target_boxes, target_present):
    f32, f16 = np.float32, np.float16
    A = np.ascontiguousarray(anchors.reshape(O, QP, 6).astype(f32, copy=False))
    pad = lambda x: np.pad(x, ((0, 0), (0, NCH * N - QP)), mode="edge")

    rc = [np.maximum(A[:, :, d], f32(0)) for d in range(3)]
    rsz = [np.maximum(A[:, :, 3 + d], f32(0)) for d in range(3)]
    alt = [pad(rc[d] - f32(0.5) * rsz[d]) for d in range(3)]
    arb = [pad(rc[d] + f32(0.5) * rsz[d]) for d in range(3)]
    rsp = [pad(rsz[d]) for d in range(3)]
    vola = pad((rsz[0] * rsz[1]) * rsz[2])
    ap6 = [pad(f32(2.5) * A[:, :, d]) for d in range(6)]
    planes = alt + arb + rsp + [vola] + ap6
    ath = np.stack([p.reshape(P, N) for p in planes]).astype(f16)
    ath = np.ascontiguousarray(ath)

    lgs = pred_logits.reshape(BS, O, QP).astype(f32, copy=False)
    lgs = np.pad(lgs, ((0, 0), (0, 0), (0, NCH * N - QP)), mode="edge")
    lg_all = np.ascontiguousarray(lgs.reshape(BS, P, N).astype(f16))

    t = target_boxes.astype(f32, copy=False)          # [BS, O, 6]
    tc_, ts_ = t[..., :3], t[..., 3:]
    blt = tc_ - f32(0.5) * ts_
    brb = tc_ + f32(0.5) * ts_
    fd = brb - blt
    volb = (fd[..., 0] * fd[..., 1]) * fd[..., 2]

    in_maps = []
    for c in range(NCORES):
        b0 = c * BL
        scv = np.zeros((BL, P, NSC), f32)
        sc3 = scv.reshape(BL, O, NCH, NSC)
        for b in range(BL):
            gb = b0 + b
            sc3[b, :, :, C_BLT : C_BLT + 3] = blt[gb][:, None, :]
            sc3[b, :, :, C_BRB : C_BRB + 3] = brb[gb][:, None, :]
            sc3[b, :, :, C_FD : C_FD + 3] = fd[gb][:, None, :]
            sc3[b, :, :, C_VOLB] = volb[gb][:, None]
            sc3[b, :, :, C_TQ : C_TQ + 6] = f32(2.5) * t[gb][:, None, :]
        in_maps.append({"ath": ath,
                        "lg": np.ascontiguousarray(lg_all[b0 : b0 + BL]),
                        "sc": scv})
    return in_maps


def _host_post(res_results, pred_logits, anchors, target_boxes,
               target_present):
    """Exact matches via f64 rescore of device candidates; labels from
    the fp16 frac planes."""
    f64 = np.float64
    A = anchors.astype(f64).reshape(O, QP, 6)
    pl = pred_logits.astype(f64).reshape(BS, O, QP)
    t = target_boxes.astype(f64)
    present = target_present.astype(bool)

    # gather candidate indices [BS, O, NCH*8]
    cand = np.empty((BS, O, NCH * 8), np.int64)
    frac = np.empty((BS, O, QP), np.float16)
    for c, r in enumerate(res_results):
        b0 = c * BL
        ix = r["ixo"].astype(np.int64).reshape(BL, O, NCH, 8)
        gq = ix + (np.arange(NCH, dtype=np.int64) * N)[None, None, :, None]
        np.clip(gq, 0, QP - 1, out=gq)
        cand[b0 : b0 + BL] = gq.reshape(BL, O, NCH * 8)
        frac[b0 : b0 + BL] = r["fr"].reshape(BL, O, NCH * N)[:, :, :QP]

    # f64 rescore with the exact reference formula
    bidx = np.arange(BS)[:, None, None]
    oidx = np.arange(O)[None, :, None]
    ab = A[oidx, cand]                                # [BS, O, K, 6]
    lgc = pl[bidx, oidx, cand]                        # [BS, O, K]
    tgt = t[:, :, None, :]                            # [BS, O, 1, 6]
    cb = np.abs(ab - tgt).sum(-1)
    cc = -1.0 / (1.0 + np.exp(-lgc))
    abx = np.clip(ab, 0.0, None)
    a_lt = abx[..., :3] - 0.5 * abx[..., 3:]
    a_rb = abx[..., :3] + 0.5 * abx[..., 3:]
    b_lt = tgt[..., :3] - 0.5 * tgt[..., 3:]
    b_rb = tgt[..., :3] + 0.5 * tgt[..., 3:]
    va = np.prod(a_rb - a_lt, -1)
    vb = np.prod(b_rb - b_lt, -1)
    it = np.prod(np.clip(np.minimum(a_rb, b_rb) - np.maximum(a_lt, b_lt),
                         0.0, None), -1)
    un = va + vb - it
    vcb = np.prod(np.clip(np.maximum(a_rb, b_rb) - np.minimum(a_lt, b_lt),
                          0.0, None), -1)
    giou = it / un - (vcb - un) / vcb
    Cc = 5.0 * cb + 2.0 * cc + 2.0 * (-giou)
    # argmin with lowest-q tie-break (reference top_k picks first index)
    order = np.lexsort((cand, Cc), axis=-1)
    best = np.take_along_axis(cand, order[..., :1], axis=-1)[..., 0]

    matches = np.zeros((BS, O, QP), np.int32)
    bo_b, bo_o = np.nonzero(present)
    matches[bo_b, bo_o, best[bo_b, bo_o]] = 1

    f = frac.astype(f64)
    fmin = f.min(-1, keepdims=True)
    fmax = f.max(-1, keepdims=True)
    sl = np.clip((f - fmin) / (fmax - fmin), 0.0, None).astype(np.float32)
    soft = np.where(present[..., None], sl, np.float32(-1.0))
    return matches, soft


def kernel(pred_logits, pred_boxes, anchors, target_boxes, target_present,
           num_top_queries):
    k = int(num_top_queries)
    assert k == 1, f"kernel specialized for num_top_queries=1, got {k}"

    if "nc" not in _BUILT:
        _BUILT["nc"] = _build_nc()
    nc = _BUILT["nc"]

    pred_logits = np.asarray(pred_logits)
    anchors = np.asarray(anchors)
    target_boxes = np.asarray(target_boxes)
    target_present = np.asarray(target_present)
    in_maps = _prep_host(pred_logits, anchors, target_boxes, target_present)
    res = run_bass_kernel_spmd(nc, in_maps, core_ids=list(range(NCORES)))
    return _host_post(res.results, pred_logits, anchors, target_boxes,
                      target_present)


# revision 8
# speedup vs baseline: 1.9237x; 1.2719x over previous
"""Trainium2 Bass kernel for the anchor-based NMS matcher (fp16 pipeline).

Math (see problem reference): per (batch b, organ o), over Qp=8192 anchor
queries q:
    cost_class = -sigmoid(logit)
    cost_bbox  = sum_d |anchor_d - tgt_d|            (cxcyczwhd space)
    cost_giou  = -giou3d(xyzxyz(clip(anchor,0)), xyzxyz(tgt))
    C = 5*cb + 2*cc + 2*cg
    matches     = one_hot(argmin_q C) * present
    soft_labels = present ? clip((cg-cgmax)/(cgmin-cgmax), 0) : -1

Device strategy (8 cores, data-parallel over batch, 2 batch items/core):
  SBUF layout: 120 partitions = (organ 20) x (q-chunk 6), free dim N=1366
  (6*1366 = 8196 = 8192 + 4 edge-pad).  All big planes are fp16: DVE
  tensor_scalar runs in 4x mode (416ns/plane) and tensor_tensor in 2x
  mode (772ns) vs 1483ns at fp32, and DMA bytes halve.

  negc = sigmoid - 2.5*cb + frac with frac = union/volc + inter/union
  (= giou + 1; affine-invariant for ranking and labels).  Work spread:
    DVE : interval endpoints, p/vc geometry, products, frac tail, top-8
    Act : sigmoid, |2.5(a-t)| via Abs, both giou reciprocals as
          exp(-ln(x)) with f32 intermediates (batched Ln then Exp to
          bound act-table reloads)
    Pool: batch-1 p planes, vc2, PSUM->SBUF negc copies
    PE  : the whole negc sum tree as PSUM-accumulated +/-identity
          matmuls (sig + frac - ab0..ab5), fp16 weights, f32 accum
  The device emits per-(b,partition) top-8 query indices of negc and the
  raw fp16 frac plane.  Host (numpy, f64) rescores the 48 candidates per
  (b,o) with the exact reference formula for exact argmin matches, and
  computes soft labels as the affine normalization of frac (absent
  organs patched to -1 on host).  Validated on the seeded data: 320/320
  candidate coverage, label l2 rel err ~7e-4.
"""

import numpy as np

import concourse.bacc as bacc
import concourse.bass as bass
import concourse.mybir as mybir
from concourse.bass_utils import run_bass_kernel_spmd
from concourse.tile import TileContext

F32 = mybir.dt.float32
F16 = mybir.dt.float16
U16 = mybir.dt.uint16
U32 = mybir.dt.uint32
ALU = mybir.AluOpType
ACTF = mybir.ActivationFunctionType
AXL = mybir.AxisListType

BS, O, QP = 16, 20, 8192
NCORES = 8
BL = BS // NCORES        # batch items per core
NCH = 6                  # q chunks per organ
N = 1366                 # chunk width; 6*1366 = 8196 = 8192 + 4 pad
P = O * NCH              # 120 partitions
NPLANES = 16             # alt0-2, arb0-2, rs0-2, vola, ap0-5 (2.5x comps)
MMW = 512                # matmul moving free-dim chunk
MMC = [(0, 512), (512, 1024), (1024, 1366)]

# sc column indices (per-partition f32 scalars, per batch item)
C_BLT = 0    # 0..2
C_BRB = 3    # 3..5
C_FD = 6     # 6..8
C_VOLB = 9
C_TQ = 10    # 10..15  (2.5 * target comps)
NSC = 16

_BUILT = {}


def _build_nc():
    nc = bacc.Bacc("TRN2", target_bir_lowering=False, debug=False)
    ath = nc.dram_tensor("ath", [NPLANES, P, N], F16, kind="ExternalInput")
    lg = nc.dram_tensor("lg", [BL, P, N], F16, kind="ExternalInput")
    sc = nc.dram_tensor("sc", [BL, P, NSC], F32, kind="ExternalInput")
    fr = nc.dram_tensor("fr", [BL, P, N], F16, kind="ExternalOutput")
    ixo = nc.dram_tensor("ixo", [BL, P, 8], U32, kind="ExternalOutput")

    from concourse.masks import make_identity

    with TileContext(nc) as tc:
        with (
            tc.tile_pool(name="big", bufs=1) as big,
            tc.tile_pool(name="sm", bufs=1) as sm,
            tc.tile_pool(name="ps", bufs=1, space="PSUM") as ps,
        ):
            # ---------------- small consts ----------------
            sct = [sm.tile([P, NSC], F32, tag=f"sct{b}", name=f"sct{b}")
                   for b in range(BL)]
            for b in range(BL):
                nc.sync.dma_start(out=sct[b][:], in_=sc[b])

            def col(b, i):
                return sct[b][:, i : i + 1]

            ipos = sm.tile([P, P], F16, tag="ipos", name="ipos")
            make_identity(nc, ipos[:])
            ineg = sm.tile([P, P], F16, tag="ineg", name="ineg")
            nc.gpsimd.memset(ineg[:], 0.0)
            nc.gpsimd.affine_select(
                out=ineg[:], in_=ineg[:],
                compare_op=ALU.not_equal, fill=-1.0, base=0,
                pattern=[[-1, P]], channel_multiplier=1)

            # ---------------- big inputs ----------------
            lgt = [big.tile([P, N], F16, tag=f"lg{b}", name=f"lg{b}")
                   for b in range(BL)]
            ain = big.tile([P, NPLANES, N], F16, tag="ain", name="ain")

            def v(j):
                return ain[:, j, :]

            ALT = [v(d) for d in range(3)]
            ARB = [v(3 + d) for d in range(3)]
            RS = [v(6 + d) for d in range(3)]
            VOLA = v(9)
            AP6 = [v(10 + d) for d in range(6)]

            def load_plane(j):
                nc.sync.dma_start(out=ain[:, j : j + 1, :],
                                  in_=ath[j : j + 1].rearrange("i p n -> p i n"))

            # first-needed-first
            load_plane(0)               # alt0
            load_plane(3)               # arb0
            for b in range(BL):
                nc.sync.dma_start(out=lgt[b][:], in_=lg[b])
            load_plane(1)               # alt1
            load_plane(4)               # arb1
            load_plane(10)              # ap0
            load_plane(11)              # ap1
            load_plane(2)               # alt2
            load_plane(5)               # arb2
            load_plane(12)              # ap2
            load_plane(13)              # ap3
            load_plane(9)               # vola
            load_plane(6)               # rs0
            load_plane(7)               # rs1
            load_plane(8)               # rs2
            load_plane(14)              # ap4
            load_plane(15)              # ap5

            # ---------------- per-batch tiles ----------------
            def bt(tag, b, dt=F16):
                return big.tile([P, N], dt, tag=f"{tag}{b}", name=f"{tag}{b}")

            SIG = [bt("sig", b) for b in range(BL)]
            MX = [[bt(f"mx{d}_", b) for d in range(3)] for b in range(BL)]
            MN = [[bt(f"mn{d}_", b) for d in range(3)] for b in range(BL)]
            PD = [[bt(f"pd{d}_", b) for d in range(3)] for b in range(BL)]
            SF = [[bt(f"sf{d}_", b) for d in range(3)] for b in range(BL)]
            MM = [[bt(f"mm{d}_", b) for d in range(3)] for b in range(BL)]
            INT = [bt("int", b) for b in range(BL)]
            VLC = [bt("vlc", b) for b in range(BL)]
            UNI = [bt("uni", b) for b in range(BL)]
            RU = [bt("ru", b) for b in range(BL)]
            RV = [bt("rv", b) for b in range(BL)]
            LNU = [bt("lnu", b, F32) for b in range(BL)]
            LNV = [bt("lnv", b, F32) for b in range(BL)]
            FRC = [bt("frc", b) for b in range(BL)]
            NG16 = [bt("ng16", b) for b in range(BL)]
            # SBUF reuse: abs planes land in MX/MN (dead after p_d);
            # vc_d lands in SF (dead after vc); XT lands in MM[0].
            VC = SF
            AB = [[MX[b][0], MX[b][1], MX[b][2],
                   MN[b][0], MN[b][1], MN[b][2]] for b in range(BL)]
            XT = [MM[b][0] for b in range(BL)]
            NGP = [ps.tile([P, 3, MMW], F32, tag=f"ngp{b}", name=f"ngp{b}")
                   for b in range(BL)]

            def mm_acc(b, plane, wt, start, stop):
                for c, (c0, c1) in enumerate(MMC):
                    nc.tensor.matmul(NGP[b][:, c, : c1 - c0], wt[:],
                                     plane[:, c0:c1], start=start, stop=stop)

            # ---------------- geometry ----------------
            # d=0 endpoints, p0 (b0 on DVE, b1 on Pool)
            for d in range(3):
                for b in range(BL):
                    nc.vector.tensor_scalar_max(out=MX[b][d][:], in0=ALT[d],
                                                scalar1=col(b, C_BLT + d))
                    nc.vector.tensor_scalar_min(out=MN[b][d][:], in0=ARB[d],
                                                scalar1=col(b, C_BRB + d))
                nc.vector.tensor_tensor(out=PD[0][d][:], in0=MN[0][d][:],
                                        in1=MX[0][d][:], op=ALU.subtract)
                nc.gpsimd.tensor_tensor(out=PD[1][d][:], in0=MN[1][d][:],
                                        in1=MX[1][d][:], op=ALU.subtract)
                if d == 0:
                    for b in range(BL):
                        nc.scalar.activation(SIG[b][:], lgt[b][:],
                                             ACTF.Sigmoid)
                if d == 1:
                    nc.vector.tensor_scalar_max(out=MM[0][0][:],
                                                in0=PD[0][0][:], scalar1=0.0)
                    for b in range(BL):
                        nc.scalar.activation(AB[b][0][:], AP6[0], ACTF.Abs,
                                             bias=col(b, C_TQ), scale=-1.0)
            # relu remaining m planes (DVE)
            nc.vector.tensor_scalar_max(out=MM[0][1][:], in0=PD[0][1][:],
                                        scalar1=0.0)
            nc.vector.tensor_scalar_max(out=MM[0][2][:], in0=PD[0][2][:],
                                        scalar1=0.0)
            for d in range(3):
                nc.vector.tensor_scalar_max(out=MM[1][d][:], in0=PD[1][d][:],
                                            scalar1=0.0)
            # Act: early abs planes; PE: sig matmuls (start groups)
            for b in range(BL):
                nc.scalar.activation(AB[b][1][:], AP6[1], ACTF.Abs,
                                     bias=col(b, C_TQ + 1), scale=-1.0)
            for b in range(BL):
                mm_acc(b, SIG[b], ipos, True, False)
                mm_acc(b, AB[b][0], ineg, False, False)
                mm_acc(b, AB[b][1], ineg, False, False)
            # DVE: inter chain + union
            for b in range(BL):
                nc.vector.tensor_tensor(out=XT[b][:], in0=MM[b][0][:],
                                        in1=MM[b][1][:], op=ALU.mult)
                nc.vector.tensor_tensor(out=INT[b][:], in0=XT[b][:],
                                        in1=MM[b][2][:], op=ALU.mult)
            for b in range(BL):
                nc.vector.tensor_scalar_add(out=UNI[b][:], in0=VOLA,
                                            scalar1=col(b, C_VOLB))
                nc.vector.tensor_tensor(out=UNI[b][:], in0=UNI[b][:],
                                        in1=INT[b][:], op=ALU.subtract)
            for b in range(BL):
                nc.scalar.activation(AB[b][2][:], AP6[2], ACTF.Abs,
                                     bias=col(b, C_TQ + 2), scale=-1.0)
            # DVE: sf/vc (vc2 on Pool), volc chain on DVE
            for d in range(3):
                for b in range(BL):
                    nc.vector.tensor_scalar_add(out=SF[b][d][:], in0=RS[d],
                                                scalar1=col(b, C_FD + d))
            for b in range(BL):
                nc.vector.tensor_tensor(out=VC[b][0][:], in0=SF[b][0][:],
                                        in1=PD[b][0][:], op=ALU.subtract)
                nc.vector.tensor_tensor(out=VC[b][1][:], in0=SF[b][1][:],
                                        in1=PD[b][1][:], op=ALU.subtract)
                nc.gpsimd.tensor_tensor(out=VC[b][2][:], in0=SF[b][2][:],
                                        in1=PD[b][2][:], op=ALU.subtract)
            for b in range(BL):
                nc.vector.tensor_tensor(out=VLC[b][:], in0=VC[b][0][:],
                                        in1=VC[b][1][:], op=ALU.mult)
            for b in range(BL):
                nc.vector.tensor_tensor(out=VLC[b][:], in0=VLC[b][:],
                                        in1=VC[b][2][:], op=ALU.mult)
            for b in range(BL):
                nc.scalar.activation(AB[b][3][:], AP6[3], ACTF.Abs,
                                     bias=col(b, C_TQ + 3), scale=-1.0)
                nc.scalar.activation(AB[b][4][:], AP6[4], ACTF.Abs,
                                     bias=col(b, C_TQ + 4), scale=-1.0)
            for b in range(BL):
                mm_acc(b, AB[b][2], ineg, False, False)
                mm_acc(b, AB[b][3], ineg, False, False)
                mm_acc(b, AB[b][4], ineg, False, False)
            # Act: batched Ln then batched Exp (bounded table reloads)
            for b in range(BL):
                nc.scalar.activation(LNU[b][:], UNI[b][:], ACTF.Ln)
            for b in range(BL):
                nc.scalar.activation(LNV[b][:], VLC[b][:], ACTF.Ln)
            for b in range(BL):
                nc.scalar.activation(RU[b][:], LNU[b][:], ACTF.Exp, scale=-1.0)
            for b in range(BL):
                nc.scalar.activation(RV[b][:], LNV[b][:], ACTF.Exp, scale=-1.0)
            # DVE: last abs plane via subtract + sign-clear (fills recip wait)
            for b in range(BL):
                nc.vector.tensor_scalar(out=AB[b][5][:], in0=AP6[5],
                                        scalar1=col(b, C_TQ + 5), scalar2=None,
                                        op0=ALU.subtract)
                nc.vector.tensor_scalar(out=AB[b][5][:].bitcast(U16),
                                        in0=AB[b][5][:].bitcast(U16),
                                        scalar1=0x7FFF, scalar2=None,
                                        op0=ALU.bitwise_and)
            for b in range(BL):
                mm_acc(b, AB[b][5], ineg, False, False)
            # DVE tail: frac = union*rv + inter*ru; PE: final accumulation
            for b in range(BL):
                nc.vector.tensor_tensor(out=XT[b][:], in0=UNI[b][:],
                                        in1=RV[b][:], op=ALU.mult)
                nc.vector.tensor_tensor(out=FRC[b][:], in0=INT[b][:],
                                        in1=RU[b][:], op=ALU.mult)
                nc.vector.tensor_tensor(out=FRC[b][:], in0=FRC[b][:],
                                        in1=XT[b][:], op=ALU.add)
                nc.sync.dma_start(out=fr[b], in_=FRC[b][:])
                mm_acc(b, FRC[b], ipos, False, True)
            # Act: PSUM -> SBUF fp16 copies (Copy is in every table set);
            # DVE: top-8
            for b in range(BL):
                for c, (c0, c1) in enumerate(MMC):
                    nc.scalar.activation(NG16[b][:, c0:c1],
                                         NGP[b][:, c, : c1 - c0], ACTF.Copy)
                mx8 = sm.tile([P, 8], F16, tag=f"mx8_{b}", name=f"mx8_{b}")
                ix8 = sm.tile([P, 8], U32, tag=f"ix8_{b}", name=f"ix8_{b}")
                nc.vector.max(out=mx8[:], in_=NG16[b][:])
                nc.vector.max_index(out=ix8[:], in_max=mx8[:],
                                    in_values=NG16[b][:])
                nc.sync.dma_start(out=ixo[b], in_=ix8[:])

    nc.finalize()
    return nc


def _prep_host(pred_logits, anchors, target_boxes, target_present):
    f32, f16 = np.float32, np.float16
    A = np.ascontiguousarray(anchors.reshape(O, QP, 6).astype(f32, copy=False))
    pad = lambda x: np.pad(x, ((0, 0), (0, NCH * N - QP)), mode="edge")

    rc = [np.maximum(A[:, :, d], f32(0)) for d in range(3)]
    rsz = [np.maximum(A[:, :, 3 + d], f32(0)) for d in range(3)]
    alt = [pad(rc[d] - f32(0.5) * rsz[d]) for d in range(3)]
    arb = [pad(rc[d] + f32(0.5) * rsz[d]) for d in range(3)]
    rsp = [pad(rsz[d]) for d in range(3)]
    vola = pad((rsz[0] * rsz[1]) * rsz[2])
    ap6 = [pad(f32(2.5) * A[:, :, d]) for d in range(6)]
    planes = alt + arb + rsp + [vola] + ap6
    ath = np.stack([p.reshape(P, N) for p in planes]).astype(f16)
    ath = np.ascontiguousarray(ath)

    lgs = pred_logits.reshape(BS, O, QP).astype(f32, copy=False)
    lgs = np.pad(lgs, ((0, 0), (0, 0), (0, NCH * N - QP)), mode="edge")
    lg_all = np.ascontiguousarray(lgs.reshape(BS, P, N).astype(f16))

    t = target_boxes.astype(f32, copy=False)          # [BS, O, 6]
    tc_, ts_ = t[..., :3], t[..., 3:]
    blt = tc_ - f32(0.5) * ts_
    brb = tc_ + f32(0.5) * ts_
    fd = brb - blt
    volb = (fd[..., 0] * fd[..., 1]) * fd[..., 2]

    in_maps = []
    for c in range(NCORES):
        b0 = c * BL
        scv = np.zeros((BL, P, NSC), f32)
        sc3 = scv.reshape(BL, O, NCH, NSC)
        for b in range(BL):
            gb = b0 + b
            sc3[b, :, :, C_BLT : C_BLT + 3] = blt[gb][:, None, :]
            sc3[b, :, :, C_BRB : C_BRB + 3] = brb[gb][:, None, :]
            sc3[b, :, :, C_FD : C_FD + 3] = fd[gb][:, None, :]
            sc3[b, :, :, C_VOLB] = volb[gb][:, None]
            sc3[b, :, :, C_TQ : C_TQ + 6] = f32(2.5) * t[gb][:, None, :]
        in_maps.append({"ath": ath,
                        "lg": np.ascontiguousarray(lg_all[b0 : b0 + BL]),
                        "sc": scv})
    return in_maps


def _host_post(res_results, pred_logits, anchors, target_boxes,
               target_present):
    """Exact matches via f64 rescore of device candidates; labels from
    the fp16 frac planes."""
    f64 = np.float64
    A = anchors.astype(f64).reshape(O, QP, 6)
    pl = pred_logits.astype(f64).reshape(BS, O, QP)
    t = target_boxes.astype(f64)
    present = target_present.astype(bool)

    # gather candidate indices [BS, O, NCH*8]
    cand = np.empty((BS, O, NCH * 8), np.int64)
    frac = np.empty((BS, O, QP), np.float16)
    for c, r in enumerate(res_results):
        b0 = c * BL
        ix = r["ixo"].astype(np.int64).reshape(BL, O, NCH, 8)
        gq = ix + (np.arange(NCH, dtype=np.int64) * N)[None, None, :, None]
        np.clip(gq, 0, QP - 1, out=gq)
        cand[b0 : b0 + BL] = gq.reshape(BL, O, NCH * 8)
        frac[b0 : b0 + BL] = r["fr"].reshape(BL, O, NCH * N)[:, :, :QP]

    # f64 rescore with the exact reference formula
    bidx = np.arange(BS)[:, None, None]
    oidx = np.arange(O)[None, :, None]
    ab = A[oidx, cand]                                # [BS, O, K, 6]
    lgc = pl[bidx, oidx, cand]                        # [BS, O, K]
    tgt = t[:, :, None, :]                            # [BS, O, 1, 6]
    cb = np.abs(ab - tgt).sum(-1)
    cc = -1.0 / (1.0 + np.exp(-lgc))
    abx = np.clip(ab, 0.0, None)
    a_lt = abx[..., :3] - 0.5 * abx[..., 3:]
    a_rb = abx[..., :3] + 0.5 * abx[..., 3:]
    b_lt = tgt[..., :3] - 0.5 * tgt[..., 3:]
    b_rb = tgt[..., :3] + 0.5 * tgt[..., 3:]
    va = np.prod(a_rb - a_lt, -1)
    vb = np.prod(b_rb - b_lt, -1)
    it = np.prod(np.clip(np.minimum(a_rb, b_rb) - np.maximum(a_lt, b_lt),
                         0.0, None), -1)
    un = va + vb - it
    vcb = np.prod(np.clip(np.maximum(a_rb, b_rb) - np.minimum(a_lt, b_lt),
                          0.0, None), -1)
    giou = it / un - (vcb - un) / vcb
    Cc = 5.0 * cb + 2.0 * cc + 2.0 * (-giou)
    # argmin with lowest-q tie-break (reference top_k picks first index)
    order = np.lexsort((cand, Cc), axis=-1)
    best = np.take_along_axis(cand, order[..., :1], axis=-1)[..., 0]

    matches = np.zeros((BS, O, QP), np.int32)
    bo_b, bo_o = np.nonzero(present)
    matches[bo_b, bo_o, best[bo_b, bo_o]] = 1

    f = frac.astype(f64)
    fmin = f.min(-1, keepdims=True)
    fmax = f.max(-1, keepdims=True)
    sl = np.clip((f - fmin) / (fmax - fmin), 0.0, None).astype(np.float32)
    soft = np.where(present[..., None], sl, np.float32(-1.0))
    return matches, soft


def kernel(pred_logits, pred_boxes, anchors, target_boxes, target_present,
           num_top_queries):
    k = int(num_top_queries)
    assert k == 1, f"kernel specialized for num_top_queries=1, got {k}"

    if "nc" not in _BUILT:
        _BUILT["nc"] = _build_nc()
    nc = _BUILT["nc"]

    pred_logits = np.asarray(pred_logits)
    anchors = np.asarray(anchors)
    target_boxes = np.asarray(target_boxes)
    target_present = np.asarray(target_present)
    in_maps = _prep_host(pred_logits, anchors, target_boxes, target_present)
    res = run_bass_kernel_spmd(nc, in_maps, core_ids=list(range(NCORES)))
    return _host_post(res.results, pred_logits, anchors, target_boxes,
                      target_present)


# revision 21
# speedup vs baseline: 1.9768x; 1.0276x over previous
"""Trainium2 Bass kernel for the anchor-based NMS matcher (fp16 pipeline).

Math (see problem reference): per (batch b, organ o), over Qp=8192 anchor
queries q:
    cost_class = -sigmoid(logit)
    cost_bbox  = sum_d |anchor_d - tgt_d|            (cxcyczwhd space)
    cost_giou  = -giou3d(xyzxyz(clip(anchor,0)), xyzxyz(tgt))
    C = 5*cb + 2*cc + 2*cg
    matches     = one_hot(argmin_q C) * present
    soft_labels = present ? clip((cg-cgmax)/(cgmin-cgmax), 0) : -1

Device strategy (8 cores, data-parallel over batch, 2 batch items/core):
  SBUF layout: 120 partitions = (organ 20) x (q-chunk 6), free dim N=1366
  (6*1366 = 8196 = 8192 + 4 edge-pad).  All big planes are fp16: DVE
  tensor_scalar runs in 4x mode (416ns/plane) and tensor_tensor in 2x
  mode (772ns) vs 1483ns at fp32, and DMA bytes halve.  Only 9 anchor
  planes are DMA'd (alt/arb endpoints + 2.5x-scaled centers); sizes
  rs = arb - alt, vola, and the scaled size planes are derived on device
  during the DMA-bound startup window.

  negc = sigmoid - 2.5*cb + frac with frac = union/volc + inter/union
  (= giou + 1; affine-invariant for ranking and labels).  Work spread:
    DVE : interval endpoints, geometry, products, frac tail, top-8
    Act : sigmoid, |2.5(a-t)| via Abs, both giou reciprocals as
          exp(-ln(x)) with f32 intermediates (batched Ln then Exp to
          bound act-table reloads), PSUM->SBUF copies
    Pool: size planes, batch-1 p planes, vc2
    PE  : the whole negc sum tree as PSUM-accumulated +/-identity
          matmuls (sig + frac - ab0..ab5), fp16 weights, f32 accum
  Top-8 runs per half-row (copies pipeline into Max), so the device
  emits 16 candidate indices per (b, partition) plus the raw fp16 frac
  plane.  Host (numpy, f64) rescores the 96 candidates per (b,o) with
  the exact reference formula for exact argmin matches, and computes
  soft labels as the affine normalization of frac (absent organs
  patched to -1 on host).  Validated on the seeded data: full candidate
  coverage, label l2 rel err ~7e-4.
"""

import numpy as np

import concourse.bacc as bacc
import concourse.bass as bass
import concourse.mybir as mybir
from concourse.bass_utils import run_bass_kernel_spmd
from concourse.tile import TileContext

F32 = mybir.dt.float32
F16 = mybir.dt.float16
U16 = mybir.dt.uint16
U32 = mybir.dt.uint32
ALU = mybir.AluOpType
ACTF = mybir.ActivationFunctionType
AXL = mybir.AxisListType

BS, O, QP = 16, 20, 8192
NCORES = 8
BL = BS // NCORES        # batch items per core
NCH = 6                  # q chunks per organ
N = 1366                 # chunk width; 6*1366 = 8196 = 8192 + 4 pad
P = O * NCH              # 120 partitions
NPLANES = 9              # alt0,arb0,alt1,arb1,alt2,arb2, ap0-2 (2.5x ctr)
MMW = 512                # matmul moving free-dim chunk
MMC = [(0, 512), (512, 1024), (1024, 1366)]
MXS = [(0, 512), (512, 1366)]   # Max/MaxIndex slices

# sc column indices (per-partition f32 scalars, per batch item)
C_BLT = 0    # 0..2
C_BRB = 3    # 3..5
C_FD = 6     # 6..8
C_VOLB = 9
C_TQ = 10    # 10..15  (2.5 * target comps)
NSC = 16

_BUILT = {}


def _build_nc():
    nc = bacc.Bacc("TRN2", target_bir_lowering=False, debug=False)
    ath = nc.dram_tensor("ath", [NPLANES, P, N], F16, kind="ExternalInput")
    lg = nc.dram_tensor("lg", [BL, P, N], F16, kind="ExternalInput")
    sc = nc.dram_tensor("sc", [BL, P, NSC], F32, kind="ExternalInput")
    fr = nc.dram_tensor("fr", [BL, P, N], F16, kind="ExternalOutput")
    ixo = nc.dram_tensor("ixo", [BL, P, 16], U32, kind="ExternalOutput")

    from concourse.masks import make_identity

    with TileContext(nc) as tc:
        with (
            tc.tile_pool(name="big", bufs=1) as big,
            tc.tile_pool(name="sm", bufs=1) as sm,
            tc.tile_pool(name="ps", bufs=1, space="PSUM") as ps,
        ):
            # ---------------- small consts ----------------
            sct = sm.tile([P, BL, NSC], F32, tag="sct", name="sct")
            nc.scalar.dma_start(out=sct[:], in_=sc.rearrange("b p i -> p b i"))

            def col(b, i):
                return sct[:, b, i : i + 1]

            ipos = sm.tile([P, P], F16, tag="ipos", name="ipos")
            make_identity(nc, ipos[:])
            ineg = sm.tile([P, P], F16, tag="ineg", name="ineg")
            nc.gpsimd.memset(ineg[:], 0.0)
            nc.gpsimd.affine_select(
                out=ineg[:], in_=ineg[:],
                compare_op=ALU.not_equal, fill=-1.0, base=0,
                pattern=[[-1, P]], channel_multiplier=1)

            # ---------------- big inputs ----------------
            lgt = [big.tile([P, N], F16, tag=f"lg{b}", name=f"lg{b}")
                   for b in range(BL)]
            ain = big.tile([P, NPLANES, N], F16, tag="ain", name="ain")

            def v(j):
                return ain[:, j, :]

            ALT = [v(0), v(2), v(4)]
            ARB = [v(1), v(3), v(5)]
            APC = [v(6), v(7), v(8)]          # 2.5x center comps

            def load_planes(j0, j1):
                nc.sync.dma_start(out=ain[:, j0:j1, :],
                                  in_=ath[j0:j1].rearrange("i p n -> p i n"))

            load_planes(0, 2)           # alt0, arb0
            for b in range(BL):
                nc.scalar.dma_start(out=lgt[b][:], in_=lg[b])
            load_planes(2, 4)           # alt1, arb1
            load_planes(4, 6)           # alt2, arb2
            load_planes(6, 7)           # ap0
            load_planes(7, 8)           # ap1
            load_planes(8, 9)           # ap2

            # ---------------- per-batch tiles ----------------
            def bt(tag, b, dt=F16):
                return big.tile([P, N], dt, tag=f"{tag}{b}", name=f"{tag}{b}")

            RSD = [big.tile([P, N], F16, tag=f"rs{d}", name=f"rs{d}")
                   for d in range(3)]
            VOLA = big.tile([P, N], F16, tag="vola", name="vola")
            SIG = [bt("sig", b) for b in range(BL)]
            MX = [[bt(f"mx{d}_", b) for d in range(3)] for b in range(BL)]
            MN = [[bt(f"mn{d}_", b) for d in range(3)] for b in range(BL)]
            PD = [[bt(f"pd{d}_", b) for d in range(3)] for b in range(BL)]
            SF = [[bt(f"sf{d}_", b) for d in range(3)] for b in range(BL)]
            MM = [[bt(f"mm{d}_", b) for d in range(3)] for b in range(BL)]
            INT = [bt("int", b) for b in range(BL)]
            VLC = [bt("vlc", b) for b in range(BL)]
            UNI = [bt("uni", b) for b in range(BL)]
            RU = [bt("ru", b) for b in range(BL)]
            RV = [bt("rv", b) for b in range(BL)]
            LND = big.tile([P, N, 4], F32, tag="lnd", name="lnd")
            FRC = [bt("frc", b) for b in range(BL)]
            NG16 = [bt("ng16", b) for b in range(BL)]
            # SBUF reuse: abs planes land in MX/MN (dead after p_d);
            # vc_d lands in SF (dead after vc); XT lands in MM[0].
            VC = SF
            AB = [[MX[b][0], MX[b][1], MX[b][2],
                   MN[b][0], MN[b][1], MN[b][2]] for b in range(BL)]
            XT = [MM[b][0] for b in range(BL)]
            NGP = [ps.tile([P, 3, MMW], F32, tag=f"ngp{b}", name=f"ngp{b}")
                   for b in range(BL)]

            def mm_acc(b, plane, wt, start, stop, chunks=MMC):
                for c0, c1 in chunks:
                    c = MMC.index((c0, c1))
                    nc.tensor.matmul(NGP[b][:, c, : c1 - c0], wt[:],
                                     plane[:, c0:c1], start=start, stop=stop)

            # ---------------- geometry ----------------
            # per-d: endpoints (DVE), rs (Pool), p (DVE b0 / Pool b1).
            # Union side first (it feeds the first Ln), volc side after.
            for d in range(3):
                for b in range(BL):
                    nc.vector.tensor_scalar_max(out=MX[b][d][:], in0=ALT[d],
                                                scalar1=col(b, C_BLT + d))
                    nc.vector.tensor_scalar_min(out=MN[b][d][:], in0=ARB[d],
                                                scalar1=col(b, C_BRB + d))
                nc.gpsimd.tensor_tensor(out=RSD[d][:], in0=ARB[d],
                                        in1=ALT[d], op=ALU.subtract)
                nc.vector.tensor_tensor(out=PD[0][d][:], in0=MN[0][d][:],
                                        in1=MX[0][d][:], op=ALU.subtract)
                nc.gpsimd.tensor_tensor(out=PD[1][d][:], in0=MN[1][d][:],
                                        in1=MX[1][d][:], op=ALU.subtract)
                if d == 0:
                    for b in range(BL):
                        nc.scalar.activation(SIG[b][:], lgt[b][:],
                                             ACTF.Sigmoid)
                if d == 1:
                    for b in range(BL):
                        nc.scalar.activation(AB[b][0][:], APC[0], ACTF.Abs,
                                             bias=col(b, C_TQ), scale=-2.5)
                if d == 2:
                    for b in range(BL):
                        nc.scalar.activation(AB[b][1][:], APC[1], ACTF.Abs,
                                             bias=col(b, C_TQ + 1),
                                             scale=-2.5)
            # Interleaved union/volc chains so both recip inputs land
            # together, just as Act drains the abs block.
            for b in range(BL):
                nc.vector.tensor_scalar_add(out=SF[b][0][:], in0=RSD[0][:],
                                            scalar1=col(b, C_FD))
                nc.vector.tensor_scalar_max(out=MM[b][0][:],
                                            in0=PD[b][0][:], scalar1=0.0)
            for b in range(BL):
                nc.vector.tensor_tensor(out=VC[b][0][:], in0=SF[b][0][:],
                                        in1=PD[b][0][:], op=ALU.subtract)
                nc.vector.tensor_scalar_max(out=MM[b][1][:],
                                            in0=PD[b][1][:], scalar1=0.0)
            for b in range(BL):
                nc.vector.tensor_scalar_add(out=SF[b][1][:], in0=RSD[1][:],
                                            scalar1=col(b, C_FD + 1))
                nc.vector.tensor_tensor(out=XT[b][:], in0=MM[b][0][:],
                                        in1=MM[b][1][:], op=ALU.mult)
            for b in range(BL):
                # vc1 on Pool so the volc product can close early
                nc.gpsimd.tensor_tensor(out=VC[b][1][:], in0=SF[b][1][:],
                                        in1=PD[b][1][:], op=ALU.subtract)
                nc.vector.tensor_scalar_max(out=MM[b][2][:],
                                            in0=PD[b][2][:], scalar1=0.0)
            for b in range(BL):
                nc.vector.tensor_tensor(out=INT[b][:], in0=XT[b][:],
                                        in1=MM[b][2][:], op=ALU.mult)
                nc.vector.tensor_scalar_add(out=SF[b][2][:], in0=RSD[2][:],
                                            scalar1=col(b, C_FD + 2))
            nc.vector.tensor_tensor(out=VOLA[:], in0=RSD[0][:],
                                    in1=RSD[1][:], op=ALU.mult)
            nc.vector.tensor_tensor(out=VOLA[:], in0=VOLA[:],
                                    in1=RSD[2][:], op=ALU.mult)
            for b in range(BL):
                nc.vector.tensor_tensor(out=VC[b][2][:], in0=SF[b][2][:],
                                        in1=PD[b][2][:], op=ALU.subtract)
                nc.vector.tensor_scalar_add(out=UNI[b][:], in0=VOLA[:],
                                            scalar1=col(b, C_VOLB))
            for b in range(BL):
                nc.vector.tensor_tensor(out=VLC[b][:], in0=VC[b][0][:],
                                        in1=VC[b][1][:], op=ALU.mult)
                nc.vector.tensor_tensor(out=UNI[b][:], in0=UNI[b][:],
                                        in1=INT[b][:], op=ALU.subtract)
            for b in range(BL):
                nc.vector.tensor_tensor(out=VLC[b][:], in0=VLC[b][:],
                                        in1=VC[b][2][:], op=ALU.mult)
            for b in range(BL):
                nc.scalar.activation(AB[b][2][:], APC[2], ACTF.Abs,
                                     bias=col(b, C_TQ + 2), scale=-2.5)
                nc.scalar.activation(AB[b][3][:], RSD[0][:], ACTF.Abs,
                                     bias=col(b, C_TQ + 3), scale=-2.5)
            for b in range(BL):
                nc.scalar.activation(AB[b][4][:], RSD[1][:], ACTF.Abs,
                                     bias=col(b, C_TQ + 4), scale=-2.5)
                nc.scalar.activation(AB[b][5][:], RSD[2][:], ACTF.Abs,
                                     bias=col(b, C_TQ + 5), scale=-2.5)
            # PE: negc accumulation for all early planes
            for b in range(BL):
                mm_acc(b, SIG[b], ipos, True, False)
                mm_acc(b, AB[b][0], ineg, False, False)
                mm_acc(b, AB[b][1], ineg, False, False)
                mm_acc(b, AB[b][2], ineg, False, False)
                mm_acc(b, AB[b][3], ineg, False, False)
                mm_acc(b, AB[b][4], ineg, False, False)
                mm_acc(b, AB[b][5], ineg, False, False)
            # Act: Ln block in readiness order (u first), then Exps
            for b in range(BL):
                nc.scalar.activation(LND[:, :, b], UNI[b][:], ACTF.Ln)
            for b in range(BL):
                nc.scalar.activation(LND[:, :, 2 + b], VLC[b][:], ACTF.Ln)
            nc.scalar.activation(RU[0][:], LND[:, :, 0], ACTF.Exp, scale=-1.0)
            nc.scalar.activation(RV[0][:], LND[:, :, 2], ACTF.Exp, scale=-1.0)
            nc.scalar.activation(RU[1][:], LND[:, :, 1], ACTF.Exp, scale=-1.0)
            nc.scalar.activation(RV[1][:], LND[:, :, 3], ACTF.Exp, scale=-1.0)
            # DVE tail: frac = union*rv + inter*ru, per-chunk PE accumulate
            # finish, Act copies, sliced top-8 (pipelined per batch)
            mx8 = [[sm.tile([P, 8], F16, tag=f"mx8_{b}_{s}",
                            name=f"mx8_{b}_{s}") for s in range(2)]
                   for b in range(BL)]
            ix8 = [sm.tile([P, 16], U32, tag=f"ix8_{b}", name=f"ix8_{b}")
                   for b in range(BL)]
            for b in range(BL):
                nc.vector.tensor_tensor(out=XT[b][:], in0=UNI[b][:],
                                        in1=RV[b][:], op=ALU.mult)
                nc.vector.tensor_tensor(out=FRC[b][:], in0=INT[b][:],
                                        in1=RU[b][:], op=ALU.mult)
                nc.vector.tensor_tensor(out=FRC[b][:], in0=FRC[b][:],
                                        in1=XT[b][:], op=ALU.add)
                nc.sync.dma_start(out=fr[b], in_=FRC[b][:])
                mm_acc(b, FRC[b], ipos, False, True)
                for c, (c0, c1) in enumerate(MMC):
                    nc.scalar.activation(NG16[b][:, c0:c1],
                                         NGP[b][:, c, : c1 - c0], ACTF.Copy)
                for s, (s0, s1) in enumerate(MXS):
                    nc.vector.max(out=mx8[b][s][:], in_=NG16[b][:, s0:s1])
                    nc.vector.max_index(out=ix8[b][:, 8 * s : 8 * s + 8],
                                        in_max=mx8[b][s][:],
                                        in_values=NG16[b][:, s0:s1])
                nc.sync.dma_start(out=ixo[b], in_=ix8[b][:])

    nc.finalize()
    return nc


def _prep_host(pred_logits, anchors, target_boxes, target_present):
    f32, f16 = np.float32, np.float16
    A = np.ascontiguousarray(anchors.reshape(O, QP, 6).astype(f32, copy=False))
    pad = lambda x: np.pad(x, ((0, 0), (0, NCH * N - QP)), mode="edge")

    rc = [np.maximum(A[:, :, d], f32(0)) for d in range(3)]
    rsz = [np.maximum(A[:, :, 3 + d], f32(0)) for d in range(3)]
    alt = [pad(rc[d] - f32(0.5) * rsz[d]) for d in range(3)]
    arb = [pad(rc[d] + f32(0.5) * rsz[d]) for d in range(3)]
    apc = [pad(A[:, :, d]) for d in range(3)]
    planes = [alt[0], arb[0], alt[1], arb[1], alt[2], arb[2]] + apc
    ath = np.stack([p.reshape(P, N) for p in planes]).astype(f16)
    ath = np.ascontiguousarray(ath)

    lgs = pred_logits.reshape(BS, O, QP).astype(f32, copy=False)
    lgs = np.pad(lgs, ((0, 0), (0, 0), (0, NCH * N - QP)), mode="edge")
    lg_all = np.ascontiguousarray(lgs.reshape(BS, P, N).astype(f16))

    t = target_boxes.astype(f32, copy=False)          # [BS, O, 6]
    tc_, ts_ = t[..., :3], t[..., 3:]
    blt = tc_ - f32(0.5) * ts_
    brb = tc_ + f32(0.5) * ts_
    fd = brb - blt
    volb = (fd[..., 0] * fd[..., 1]) * fd[..., 2]

    in_maps = []
    for c in range(NCORES):
        b0 = c * BL
        scv = np.zeros((BL, P, NSC), f32)
        sc3 = scv.reshape(BL, O, NCH, NSC)
        for b in range(BL):
            gb = b0 + b
            sc3[b, :, :, C_BLT : C_BLT + 3] = blt[gb][:, None, :]
            sc3[b, :, :, C_BRB : C_BRB + 3] = brb[gb][:, None, :]
            sc3[b, :, :, C_FD : C_FD + 3] = fd[gb][:, None, :]
            sc3[b, :, :, C_VOLB] = volb[gb][:, None]
            sc3[b, :, :, C_TQ : C_TQ + 6] = f32(2.5) * t[gb][:, None, :]
        in_maps.append({"ath": ath,
                        "lg": np.ascontiguousarray(lg_all[b0 : b0 + BL]),
                        "sc": scv})
    return in_maps


def _host_post(res_results, pred_logits, anchors, target_boxes,
               target_present):
    """Exact matches via f64 rescore of device candidates; labels from
    the fp16 frac planes."""
    f64 = np.float64
    A = anchors.astype(f64).reshape(O, QP, 6)
    pl = pred_logits.astype(f64).reshape(BS, O, QP)
    t = target_boxes.astype(f64)
    present = target_present.astype(bool)

    # candidate q indices: per (b, organ, chunk) two 8-cand slices with
    # free-dim offsets 0 and 512
    K = NCH * 16
    cand = np.empty((BS, O, K), np.int64)
    frac = np.empty((BS, O, QP), np.float16)
    soff = np.array([s0 for s0, _ in MXS], np.int64)
    for c, r in enumerate(res_results):
        b0 = c * BL
        ix = r["ixo"].astype(np.int64).reshape(BL, O, NCH, 2, 8)
        gq = (ix + soff[None, None, None, :, None]
              + (np.arange(NCH, dtype=np.int64) * N)[None, None, :, None,
                                                     None])
        np.clip(gq, 0, QP - 1, out=gq)
        cand[b0 : b0 + BL] = gq.reshape(BL, O, K)
        frac[b0 : b0 + BL] = r["fr"].reshape(BL, O, NCH * N)[:, :, :QP]

    # f64 rescore with the exact reference formula
    bidx = np.arange(BS)[:, None, None]
    oidx = np.arange(O)[None, :, None]
    ab = A[oidx, cand]                                # [BS, O, K, 6]
    lgc = pl[bidx, oidx, cand]                        # [BS, O, K]
    tgt = t[:, :, None, :]                            # [BS, O, 1, 6]
    cb = np.abs(ab - tgt).sum(-1)
    cc = -1.0 / (1.0 + np.exp(-lgc))
    abx = np.clip(ab, 0.0, None)
    a_lt = abx[..., :3] - 0.5 * abx[..., 3:]
    a_rb = abx[..., :3] + 0.5 * abx[..., 3:]
    b_lt = tgt[..., :3] - 0.5 * tgt[..., 3:]
    b_rb = tgt[..., :3] + 0.5 * tgt[..., 3:]
    va = np.prod(a_rb - a_lt, -1)
    vb = np.prod(b_rb - b_lt, -1)
    it = np.prod(np.clip(np.minimum(a_rb, b_rb) - np.maximum(a_lt, b_lt),
                         0.0, None), -1)
    un = va + vb - it
    vcb = np.prod(np.clip(np.maximum(a_rb, b_rb) - np.minimum(a_lt, b_lt),
                          0.0, None), -1)
    giou = it / un - (vcb - un) / vcb
    Cc = 5.0 * cb + 2.0 * cc + 2.0 * (-giou)
    # argmin with lowest-q tie-break (reference top_k picks first index)
    order = np.lexsort((cand, Cc), axis=-1)
    best = np.take_along_axis(cand, order[..., :1], axis=-1)[..., 0]

    matches = np.zeros((BS, O, QP), np.int32)
    bo_b, bo_o = np.nonzero(present)
    matches[bo_b, bo_o, best[bo_b, bo_o]] = 1

    f = frac.astype(f64)
    fmin = f.min(-1, keepdims=True)
    fmax = f.max(-1, keepdims=True)
    sl = np.clip((f - fmin) / (fmax - fmin), 0.0, None).astype(np.float32)
    soft = np.where(present[..., None], sl, np.float32(-1.0))
    return matches, soft


def kernel(pred_logits, pred_boxes, anchors, target_boxes, target_present,
           num_top_queries):
    k = int(num_top_queries)
    assert k == 1, f"kernel specialized for num_top_queries=1, got {k}"

    if "nc" not in _BUILT:
        _BUILT["nc"] = _build_nc()
    nc = _BUILT["nc"]

    pred_logits = np.asarray(pred_logits)
    anchors = np.asarray(anchors)
    target_boxes = np.asarray(target_boxes)
    target_present = np.asarray(target_present)
    in_maps = _prep_host(pred_logits, anchors, target_boxes, target_present)
    res = run_bass_kernel_spmd(nc, in_maps, core_ids=list(range(NCORES)))
    return _host_post(res.results, pred_logits, anchors, target_boxes,
                      target_present)
